# revision 36
# baseline (speedup 1.0000x reference)
"""Trainium2 Bass kernel for a dense transformer block (nn_Block_3453153706485).

B=4, S=1024, D=1024, H=16 heads (hd=64), FF=4096, fp32 I/O.
Sharding: 8 cores; core c owns (batch b=c//2, token half c%2) -> 512 query
tokens.  Keys are mask-compressed: only unmasked key tokens (padded to a
multiple of 256) are projected / attended, since masked keys contribute
exp(-inf)=0 to softmax.

Per-core pipeline:
  A:  LN1 of the 4 local query tiles (xq) and kpt gathered key tiles (xk);
      h (bf16) -> PE-transpose -> hqT/hkT fp8 [D, tok].
  B:  q/k/v projections via fp8 DoubleRow matmuls against 32x-scaled fp8
      weights; RoPE (tables pre-scaled by 1/32) via one psum->bf16 eviction
      + two Pool multiplies + Pool add -> kr/qr bf16 -> PE-transpose ->
      kT/qT bf16.  v is evicted (psum->fp8, kept 32x scaled) into
      v1 [ktok, skc, head, 66] with col 64 = 32.0 (the 32-scaled softmax
      denominator row).
  C1: per key tile skc, per head pair j: scoresT = kT_h.T @ qT_h (bf16);
      exp(s/8 + mask_bias) -> pT fp8 [ktok, skc, head, q].
  C2: PV in [q, d] orientation: out[q, head, 0:66] += pT.T @ v1 (DR);
      col 64 = 32Z.  Normalize with a per-(q,head) reciprocal broadcast
      along the free dim, -> attn bf16 -> PE-transpose -> attnT fp8.
  D:  wo (DR) + residual via scalar_tensor_tensor (x 1/32 folded in);
      LN2 -> h2 bf16 -> transpose -> h2Tb bf16; fp8 split h2Th = fp8(h2Tb),
      h2Tl = fp8(8*(h2Tb - h2Th)) for the compensated fc1.
  E:  fc1 = h2Th@w1hi + h2Th@(w1lo/8) + h2Tl@(w1hi/8), three fp8 DR chains
      into one psum (32x scaled); w1 chunks streamed just-in-time from one
      interleaved DRAM tensor.  gelu applies the 1/32 descale -> g1 fp8;
      fc2 fp8 DR; final residual via scalar_tensor_tensor.
"""

from contextlib import ExitStack

import ml_dtypes
import numpy as np

import concourse.bass as bass
import concourse.tile as tile
from concourse import bacc, mybir
from concourse.masks import make_identity

F32 = mybir.dt.float32
BF16 = mybir.dt.bfloat16
F8 = mybir.dt.float8e4
AF = mybir.ActivationFunctionType
OP = mybir.AluOpType
DR = mybir.MatmulPerfMode.DoubleRow

P = 128
D = 1024
H = 16
HD = 64
FF = 4096
FB = FF // P       # 32 fc1 output blocks
TB = 1024          # tokens per batch
TL = 512           # local (query) tokens per core
QT = TL // P       # 4 query tiles
KC = D // P        # 8
NCORES = 8
EPS = 1e-5
WS = 32.0          # fp8 weight upscale
RWS = float(1.0 / WS)
MBIAS = -30.0      # exp bias for padding keys


def _swap_pairs(ap4):
    """View with the two elements of each innermost [step,2] pair swapped."""
    st = ap4.ap[-1][0]
    return bass.AP(
        tensor=ap4.tensor,
        offset=ap4.offset + st,
        ap=list(ap4.ap[:-1]) + [[-st, 2]],
    )


def build_program(kpt: int, apply_ln1: bool, apply_ln2: bool,
                  sim_compat: bool = False, dbg: bool = False):
    assert kpt % 2 == 0 and 2 <= kpt <= 8
    KP = kpt * P
    nc = bacc.Bacc("TRN2", target_bir_lowering=False, debug=False)

    xq_d = nc.dram_tensor("xq", [TL, D], F32, kind="ExternalInput").ap()
    xk_d = nc.dram_tensor("xk", [KP, D], F32, kind="ExternalInput").ap()
    mb_d = nc.dram_tensor("mb", [P, kpt], F32, kind="ExternalInput").ap()
    cosq_d = nc.dram_tensor("cosq", [P, QT, HD], F32, kind="ExternalInput").ap()
    sinq_d = nc.dram_tensor("sinq", [P, QT, HD], F32, kind="ExternalInput").ap()
    cosk_d = nc.dram_tensor("cosk", [P, kpt, HD], F32, kind="ExternalInput").ap()
    sink_d = nc.dram_tensor("sink", [P, kpt, HD], F32, kind="ExternalInput").ap()
    wq_d = nc.dram_tensor("wq", [P, KC, D], F8, kind="ExternalInput").ap()
    wk_d = nc.dram_tensor("wk", [P, KC, D], F8, kind="ExternalInput").ap()
    wv_d = nc.dram_tensor("wv", [P, KC, D], F8, kind="ExternalInput").ap()
    wo_d = nc.dram_tensor("wo", [P, KC, D], F8, kind="ExternalInput").ap()
    # fc1 weights, fb-major bf16 chunks for just-in-time streaming
    w1a_d = nc.dram_tensor("w1a", [P, FB, KC, P], BF16,
                           kind="ExternalInput").ap()
    w2_d = nc.dram_tensor("w2", [P, FF // P, D], F8, kind="ExternalInput").ap()
    ln_d = {nm: nc.dram_tensor(nm, [1, D], F32, kind="ExternalInput").ap()
            for nm in ("ln1w", "ln1b", "ln2w", "ln2b")}
    out_d = nc.dram_tensor("out", [TL, D], F32, kind="ExternalOutput").ap()
    dbg_d = {}
    if dbg:
        for nm, shape, dt in [
                ("d_hqT", [P, KC, TL], F8), ("d_hkT", [P, KC, KP], F8),
                ("d_kT", [P, KC, KP], BF16), ("d_qT", [P, KC, TL], BF16),
                ("d_v1", [P, kpt, H, 66], F8),
                ("d_at", [P, KC, TL], F8), ("d_xr", [P, QT, D], F32),
                ("d_g1", [P, FF // P, TL], F8), ("d_araw", [P, QT, H, 66], F32)]:
            dbg_d[nm] = nc.dram_tensor(nm, shape, dt,
                                       kind="ExternalOutput").ap()

    gelu_f = AF.Identity if sim_compat else AF.Gelu

    with tile.TileContext(nc) as tc:
        es0 = ExitStack()

        # ---- left pools (live to program end, released in reverse) ----
        cons = es0.enter_context(tc.tile_pool(name="cons", bufs=1))
        work = es0.enter_context(tc.tile_pool(name="work", bufs=2))
        pool_xq = tc.alloc_tile_pool(name="p_xq", bufs=1)
        xq_sb = pool_xq.tile([P, QT, D], F32, name="xq_sb")
        pool_kq = tc.alloc_tile_pool(name="p_kq", bufs=1)
        qT = pool_kq.tile([P, KC, TL], BF16, name="qT")
        kT = pool_kq.tile([P, KC, KP], BF16, name="kT")
        pool_v1 = tc.alloc_tile_pool(name="p_v1", bufs=1)
        v1 = pool_v1.tile([P, kpt, H, 66], F8, name="v1")

        # ---- right pools; stack order encodes release order ----
        pool_w2 = tc.alloc_tile_pool(name="p_w2", bufs=1, side="right")
        w2_sb = pool_w2.tile([P, FF // P, D], F8, name="w2_sb")
        pool_wD = tc.alloc_tile_pool(name="p_wD", bufs=1, side="right")
        wo_sb = pool_wD.tile([P, KC, D], F8, name="wo_sb")
        pool_pT = tc.alloc_tile_pool(name="p_pT", bufs=1, side="right")
        pool_hT = tc.alloc_tile_pool(name="p_hT", bufs=1, side="right")
        hqT = pool_hT.tile([P, KC, TL], F8, name="hqT")
        hkT = pool_hT.tile([P, KC, KP], F8, name="hkT")
        workB = tc.alloc_tile_pool(name="workB", bufs=2, side="right")
        pool_wB = tc.alloc_tile_pool(name="p_wB", bufs=1, side="right")
        wq_sb = pool_wB.tile([P, KC, D], F8, name="wq_sb")
        wk_sb = pool_wB.tile([P, KC, D], F8, name="wk_sb")
        wv_sb = pool_wB.tile([P, KC, D], F8, name="wv_sb")

        # ---------------- constants + weight DMAs ----------------
        identb = cons.tile([P, P], BF16, name="identb")
        make_identity(nc, identb)
        eps_t = cons.tile([P, 1], F32, name="eps_t")
        nc.vector.memset(eps_t[:], EPS)
        mbias = cons.tile([P, kpt], F32, name="mb_sb")
        nc.gpsimd.dma_start(mbias[:], mb_d)
        cosq = cons.tile([P, QT, HD], F32, name="cosq_sb")
        nc.gpsimd.dma_start(cosq[:], cosq_d)
        sinq = cons.tile([P, QT, HD], F32, name="sinq_sb")
        nc.gpsimd.dma_start(sinq[:], sinq_d)
        cosk = cons.tile([P, kpt, HD], F32, name="cosk_sb")
        nc.gpsimd.dma_start(cosk[:], cosk_d)
        sink = cons.tile([P, kpt, HD], F32, name="sink_sb")
        nc.gpsimd.dma_start(sink[:], sink_d)

        ln_bc = {}
        for nm, need in (("ln1w", apply_ln1), ("ln1b", apply_ln1),
                         ("ln2w", apply_ln2), ("ln2b", apply_ln2)):
            if need:
                t = cons.tile([P, D], F32, name=f"{nm}_bc")
                src = bass.AP(tensor=ln_d[nm].tensor, offset=ln_d[nm].offset,
                              ap=[[0, P], [1, D]])
                nc.gpsimd.dma_start(t[:], src)
                ln_bc[nm] = t

        # q/k/v weights on the gpsimd SWDGE queue (x tiles own the sync
        # queue; the ACT queue must stay clear for the LN sqrt/apply chain)
        nc.gpsimd.dma_start(wq_sb[:], wq_d)
        nc.gpsimd.dma_start(wk_sb[:], wk_d)
        nc.gpsimd.dma_start(wv_sb[:], wv_d)

        nc.vector.memset(v1[:, :, :, 64:65], WS)
        nc.vector.memset(v1[:, :, :, 65:66], 0.0)

        # psum pool for A/B/C1: pj f32 [P,D] (2 banks) x3, tpb bf16 x2
        psAB = tc.alloc_tile_pool(name="psAB", bufs=1, space="PSUM")

        def layernorm(src_ap, dst_ap, wkey, bkey, applied, eng="act"):
            stats = work.tile([P, 2, 6], F32, tag="stats", name="stats")
            nc.vector.bn_stats(stats[:, 0, :], src_ap[:, 0:512])
            nc.vector.bn_stats(stats[:, 1, :], src_ap[:, 512:1024])
            mv = work.tile([P, 2], F32, tag="mv", name="mv")
            nc.vector.bn_aggr(mv[:], stats[:])
            std = work.tile([P, 1], F32, tag="std", name="std")
            nc.scalar.activation(std[:], mv[:, 1:2], AF.Sqrt, bias=eps_t[:])
            rstd = work.tile([P, 1], F32, tag="rstd", name="rstd")
            nc.vector.reciprocal(rstd[:], std[:])
            if eng == "act":
                # (x - m)*rstd == Identity(rstd*x + (-m*rstd)) on ACT
                nmr = work.tile([P, 1], F32, tag="nmr", name="nmr")
                nc.vector.tensor_scalar(nmr[:], mv[:, 0:1], rstd[:], -1.0,
                                        OP.mult, OP.mult)
                nc.scalar.activation(dst_ap, src_ap, AF.Identity,
                                     scale=rstd[:], bias=nmr[:])
            else:
                nc.vector.tensor_scalar(dst_ap, src_ap, mv[:, 0:1], rstd[:],
                                        OP.subtract, OP.mult)
            if applied:
                nc.gpsimd.tensor_mul(dst_ap, dst_ap, ln_bc[wkey][:])
                nc.gpsimd.tensor_add(dst_ap, dst_ap, ln_bc[bkey][:])

        def transpose8(pool, h_ap, dst_ap, evict_eng):
            """h_ap [128, 1024] bf16 -> dst_ap [128, 8, 128] (cast on evict)."""
            pst = pool.tile([P, D], BF16, tag="tpb", bufs=2, name="pst")
            for c in range(KC):
                nc.tensor.transpose(pst[:, c * P:(c + 1) * P],
                                    h_ap[:, c * P:(c + 1) * P], identb[:])
            evict_eng(dst_ap, pst.rearrange("p (k t) -> p k t", k=KC))

        def proj_dr(ps_half, lhs3, w_sb, nh):
            for k2 in range(KC // 2):
                nc.tensor.matmul(
                    ps_half, lhsT=lhs3[:, 2 * k2:2 * k2 + 2, :],
                    rhs=w_sb[:, 2 * k2:2 * k2 + 2, nh * 512:(nh + 1) * 512],
                    start=(k2 == 0), stop=(k2 == KC // 2 - 1),
                    perf_mode=DR)

        def rope(ps, cos_sb, sin_sb, ti, kr_ap, add_eng):
            """psum [128,1024] f32 -> kr_ap bf16, rotated, 1/32 descale in
            the tables.  Two DVE psum reads, Pool add."""
            ps_h = ps.rearrange("p (h i) -> p h i", h=H)
            cos_b = cos_sb[:, ti, None, :].to_broadcast((P, H, HD))
            p1 = workB.tile([P, D], BF16, tag="p1", bufs=2, name=f"p1{ti}")
            nc.vector.tensor_tensor(p1.rearrange("p (h i) -> p h i", h=H),
                                    ps_h, cos_b, OP.mult)
            ps_sw = _swap_pairs(ps.rearrange("p (h i two) -> p h i two",
                                             h=H, two=2))
            sin_b = (sin_sb[:, ti, None, :].to_broadcast((P, H, HD))
                     .rearrange("p h (i two) -> p h i two", two=2))
            p2 = workB.tile([P, D], BF16, tag="p2", bufs=2, name=f"p2{ti}")
            nc.vector.tensor_tensor(
                p2.rearrange("p (h i two) -> p h i two", h=H, two=2),
                ps_sw, sin_b, OP.mult)
            add_eng(kr_ap, p1[:], p2[:])

        # prefetch all key tiles (scalar queue; the sync HWDGE carries xq)
        xks = []
        for ti in range(kpt):
            xk = workB.tile([P, D], F32, tag="xk", bufs=kpt, name=f"xk{ti}")
            nc.scalar.dma_start(xk[:], xk_d[ti * P:(ti + 1) * P, :])
            xks.append(xk)

        # ========== stage A+B: q group then k group, each stage-major ====
        # Group-local stage-major emission: deep pipelining within a group,
        # and q work is never queued behind k-tile DMA arrival.
        NT = QT + kpt

        def srcx(t):
            return xq_sb[:, t, :] if t < QT else xks[t - QT][:]

        def hT_slice(t):
            if t < QT:
                return hqT[:, :, t * P:(t + 1) * P]
            return hkT[:, :, (t - QT) * P:(t - QT + 1) * P]

        for t in range(QT):
            nc.sync.dma_start(xq_sb[:, t, :], xq_d[t * P:(t + 1) * P, :])
        # late-phase weights behind the x tiles on the sync queue
        nc.sync.dma_start(wo_sb[:], wo_d)
        nc.sync.dma_start(w2_sb[:], w2_d)

        rstds = [None] * NT

        def ln_stats(t):
            stats = work.tile([P, 2, 6], F32, tag="stA", bufs=3,
                              name=f"st{t}")
            nc.vector.bn_stats(stats[:, 0, :], srcx(t)[:, 0:512])
            nc.vector.bn_stats(stats[:, 1, :], srcx(t)[:, 512:1024])
            mv = work.tile([P, 2], F32, tag="mvA", bufs=3, name=f"mv{t}")
            nc.vector.bn_aggr(mv[:], stats[:])
            std = work.tile([P, 1], F32, tag="stdA", bufs=3, name=f"std{t}")
            nc.scalar.activation(std[:], mv[:, 1:2], AF.Sqrt, bias=eps_t[:])
            rs = work.tile([P, 2], F32, tag="rsA", bufs=NT, name=f"rs{t}")
            nc.vector.reciprocal(rs[:, 0:1], std[:])
            nc.vector.tensor_scalar(rs[:, 1:2], mv[:, 0:1], rs[:, 0:1], -1.0,
                                    OP.mult, OP.mult)
            rstds[t] = rs

        def ln_apply(t):
            rs = rstds[t]
            h = work.tile([P, D], BF16, tag="h", bufs=3, name=f"h{t}")
            nc.scalar.activation(h[:], srcx(t), AF.Identity,
                                 scale=rs[:, 0:1], bias=rs[:, 1:2])
            if apply_ln1:
                nc.gpsimd.tensor_mul(h[:], h[:], ln_bc["ln1w"][:])
                nc.gpsimd.tensor_add(h[:], h[:], ln_bc["ln1b"][:])
            transpose8(psAB, h[:], hT_slice(t), nc.scalar.copy)

        def proj_rope(t):
            isq = t < QT
            w_sb = wq_sb if isq else wk_sb
            ps = psAB.tile([P, D], F32, tag="pj", bufs=3, name=f"pj{t}")
            for nh in range(2):
                proj_dr(ps[:, nh * 512:(nh + 1) * 512], hT_slice(t),
                        w_sb, nh)
            kr = workB.tile([P, D], BF16, tag="kr", bufs=2, name=f"kr{t}")
            if isq:
                rope(ps, cosq, sinq, t, kr[:], nc.vector.tensor_add)
                transpose8(psAB, kr[:], qT[:, :, t * P:(t + 1) * P],
                           nc.scalar.copy)
            else:
                tk = t - QT
                rope(ps, cosk, sink, tk, kr[:], nc.gpsimd.tensor_add)
                transpose8(psAB, kr[:], kT[:, :, tk * P:(tk + 1) * P],
                           nc.scalar.copy)

        for t in range(QT):
            ln_stats(t)
        for t in range(QT):
            ln_apply(t)
        for t in range(QT):
            proj_rope(t)
        for t in range(QT, NT):
            ln_stats(t)
        for t in range(QT, NT):
            ln_apply(t)
        for t in range(QT, NT):
            proj_rope(t)

        # v projections -> v1 (kept 32x scaled); evict on DVE so the ACT
        # queue is free to start the exp phase immediately
        for tk in range(kpt):
            psv = psAB.tile([P, D], F32, tag="pj", bufs=3, name=f"vps{tk}")
            for nh in range(2):
                proj_dr(psv[:, nh * 512:(nh + 1) * 512],
                        hkT[:, :, tk * P:(tk + 1) * P], wv_sb, nh)
            nc.vector.tensor_copy(v1[:, tk, :, 0:64],
                                  psv.rearrange("p (h d) -> p h d", h=H))

        if dbg:
            nc.sync.dma_start(dbg_d["d_hqT"], hqT[:])
            nc.sync.dma_start(dbg_d["d_hkT"], hkT[:])
            nc.sync.dma_start(dbg_d["d_kT"], kT[:])
            nc.sync.dma_start(dbg_d["d_qT"], qT[:])
            nc.sync.dma_start(dbg_d["d_v1"], v1[:])

        pool_wB.release()
        workB.release()
        pool_hT.release()

        # ========== stage C: per head pair: scores, exp, PV, normalize ====
        psAB.release()
        psC2 = tc.alloc_tile_pool(name="psC2", bufs=1, space="PSUM")
        pool_at = tc.alloc_tile_pool(name="p_at", bufs=1)
        attnT = pool_at.tile([P, KC, TL], F8, name="attnT")
        attn_sb = pool_at.tile([P, QT, H, HD], BF16, name="attn_sb")

        def pv_finish(j, pTj):
            pv = psC2.tile([P, QT, 2, P], F32, tag="pv", bufs=2,
                           name=f"pv{j}")
            for qb in range(QT):
                for hhi in range(2):
                    for sp in range(kpt // 2):
                        nc.tensor.matmul(
                            pv[:, qb, hhi, 0:66],
                            lhsT=pTj[:, 2 * sp:2 * sp + 2, hhi,
                                     qb * P:(qb + 1) * P],
                            rhs=v1[:, 2 * sp:2 * sp + 2, 2 * j + hhi, :],
                            start=(sp == 0), stop=(sp == kpt // 2 - 1),
                            perf_mode=DR)
            araw = work.tile([P, QT, 2, 66], F32, tag="araw", bufs=2,
                             name=f"araw{j}")
            nc.vector.tensor_copy(araw[:], pv[:, :, :, 0:66])
            if dbg:
                nc.sync.dma_start(dbg_d["d_araw"][:, :, 2 * j:2 * j + 2, :],
                                  araw[:])
            rz = work.tile([P, QT, 2], F32, tag="rz", bufs=2, name=f"rz{j}")
            nc.vector.reciprocal(rz[:], araw[:, :, :, 64])
            nc.gpsimd.tensor_mul(
                attn_sb[:, :, 2 * j:2 * j + 2, :], araw[:, :, :, 0:64],
                rz[:, :, :, None].to_broadcast((P, QT, 2, HD)))

        prev = None
        for j in range(H // 2):
            pTj = pool_pT.tile([P, kpt, 2, TL], F8, tag="pTr", bufs=2,
                               name=f"pT{j}")
            for skc in range(kpt):
                pss = psC2.tile([P, D], F32, tag="sc", bufs=2,
                                name=f"sc{j}_{skc}")
                for eo in range(2):
                    pb = 64 * eo
                    nc.tensor.matmul(
                        pss[:, eo * 512:(eo + 1) * 512],
                        lhsT=kT[pb:pb + 64, j, skc * P:(skc + 1) * P],
                        rhs=qT[pb:pb + 64, j, :],
                        start=True, stop=True)
                nc.scalar.activation(
                    pTj[:, skc, :, :], pss[:], AF.Exp,
                    scale=0.125, bias=mbias[:, skc:skc + 1])
            if prev is not None:
                pv_finish(*prev)
            prev = (j, pTj)
        pv_finish(*prev)

        psC2.release()
        psD = tc.alloc_tile_pool(name="psD", bufs=1, space="PSUM")
        pool_w1s = tc.alloc_tile_pool(name="p_w1s", bufs=1, side="right")

        def transpose8d(h_ap, dst_ap, evict_eng):
            transpose8(psD, h_ap, dst_ap, evict_eng)

        for qb in range(QT):
            transpose8d(attn_sb[:, qb, :, :].rearrange("p h d -> p (h d)"),
                        attnT[:, :, qb * P:(qb + 1) * P], nc.scalar.copy)

        # ========== stage D: wo + residual, LN2, h2T hi/lo ==========
        pool_res = tc.alloc_tile_pool(name="p_res", bufs=1)
        xres = pool_res.tile([P, QT, D], F32, name="xres")
        h2Tb = pool_res.tile([P, KC, TL], BF16, name="h2Tb")

        # stage-major: all wo+residuals, then all LN2 stats, then all applies
        for tc4 in range(QT):
            for nh in range(2):
                ps = psD.tile([P, 512], F32, tag="wo", bufs=3,
                              name=f"wops{tc4}_{nh}")
                for k2 in range(KC // 2):
                    nc.tensor.matmul(
                        ps[:], lhsT=attnT[:, 2 * k2:2 * k2 + 2,
                                          tc4 * P:(tc4 + 1) * P],
                        rhs=wo_sb[:, 2 * k2:2 * k2 + 2,
                                  nh * 512:(nh + 1) * 512],
                        start=(k2 == 0), stop=(k2 == KC // 2 - 1),
                        perf_mode=DR)
                nc.vector.scalar_tensor_tensor(
                    xres[:, tc4, nh * 512:(nh + 1) * 512], ps[:], RWS,
                    xq_sb[:, tc4, nh * 512:(nh + 1) * 512],
                    OP.mult, OP.add)
        rs2 = []
        for tc4 in range(QT):
            stats = work.tile([P, 2, 6], F32, tag="stA", bufs=3,
                              name=f"st2{tc4}")
            nc.vector.bn_stats(stats[:, 0, :], xres[:, tc4, 0:512])
            nc.vector.bn_stats(stats[:, 1, :], xres[:, tc4, 512:1024])
            mv = work.tile([P, 2], F32, tag="mvA", bufs=3, name=f"mv2{tc4}")
            nc.vector.bn_aggr(mv[:], stats[:])
            std = work.tile([P, 1], F32, tag="stdA", bufs=3,
                            name=f"std2{tc4}")
            nc.scalar.activation(std[:], mv[:, 1:2], AF.Sqrt, bias=eps_t[:])
            rs = work.tile([P, 2], F32, tag="rs2", bufs=QT, name=f"rs2{tc4}")
            nc.vector.reciprocal(rs[:, 0:1], std[:])
            nc.vector.tensor_scalar(rs[:, 1:2], mv[:, 0:1], rs[:, 0:1], -1.0,
                                    OP.mult, OP.mult)
            rs2.append(rs)
        for tc4 in range(QT):
            h2 = work.tile([P, D], BF16, tag="h", bufs=3, name=f"h2{tc4}")
            nc.scalar.activation(h2[:], xres[:, tc4, :], AF.Identity,
                                 scale=rs2[tc4][:, 0:1], bias=rs2[tc4][:, 1:2])
            if apply_ln2:
                nc.gpsimd.tensor_mul(h2[:], h2[:], ln_bc["ln2w"][:])
                nc.gpsimd.tensor_add(h2[:], h2[:], ln_bc["ln2b"][:])
            transpose8d(h2[:], h2Tb[:, :, tc4 * P:(tc4 + 1) * P],
                        nc.vector.tensor_copy)

        if dbg:
            nc.sync.dma_start(dbg_d["d_at"], attnT[:])
            nc.sync.dma_start(dbg_d["d_xr"], xres[:])

        psD.release()

        # ========== stage E: MLP ==========
        psE = tc.alloc_tile_pool(name="psE", bufs=1, space="PSUM")
        pool_g1 = tc.alloc_tile_pool(name="p_g1", bufs=1)
        g1 = pool_g1.tile([P, FF // P, TL], F8, name="g1")

        for g in range(FB // 2):
            w1s = pool_w1s.tile([P, 2, KC, P], BF16, tag="w1s", bufs=3,
                                name=f"w1s{g}")
            weng = nc.sync if g % 2 == 0 else nc.scalar
            weng.dma_start(w1s[:], w1a_d[:, 2 * g:2 * g + 2, :, :])
            ps = psE.tile([P, 2, 512], F32, tag="m1", bufs=2,
                          name=f"m1ps{g}")
            for i in range(2):
                for c in range(KC):
                    nc.tensor.matmul(
                        ps[:, i, :],
                        lhsT=w1s[:, i, c, :],
                        rhs=h2Tb[:, c, :],
                        start=(c == 0), stop=(c == KC - 1))
            nc.scalar.activation(
                g1[:, 2 * g:2 * g + 2, :].rearrange("p a b -> p (a b)"),
                ps.rearrange("p a b -> p (a b)"), gelu_f)

        for nh in range(2):
            for tc4 in range(QT):
                ps = psE.tile([P, 512], F32, tag="m2", bufs=2,
                              name=f"m2ps{nh}_{tc4}")
                for kp2 in range(FF // 256):
                    nc.tensor.matmul(
                        ps[:],
                        lhsT=g1[:, 2 * kp2:2 * kp2 + 2,
                                tc4 * P:(tc4 + 1) * P],
                        rhs=w2_sb[:, 2 * kp2:2 * kp2 + 2,
                                  nh * 512:(nh + 1) * 512],
                        start=(kp2 == 0), stop=(kp2 == FF // 256 - 1),
                        perf_mode=DR)
                ot = work.tile([P, 512], F32, tag="osb", name=f"ot{nh}_{tc4}")
                nc.vector.scalar_tensor_tensor(
                    ot[:], ps[:], RWS,
                    xres[:, tc4, nh * 512:(nh + 1) * 512], OP.mult, OP.add)
                oeng = nc.sync if tc4 % 2 == 0 else nc.scalar
                oeng.dma_start(out_d[tc4 * P:(tc4 + 1) * P,
                                     nh * 512:(nh + 1) * 512], ot[:])

        if dbg:
            nc.sync.dma_start(dbg_d["d_g1"], g1[:])

        pool_g1.release()
        pool_res.release()
        pool_at.release()
        pool_v1.release()
        pool_kq.release()
        pool_xq.release()
        pool_w1s.release()
        pool_pT.release()
        pool_wD.release()
        pool_w2.release()
        psE.release()
        es0.close()

    nc.compile()
    return nc


# ---------------------------------------------------------------------------
# Host side
# ---------------------------------------------------------------------------

_PROGRAM_CACHE = {}


def _get_program(kpt, apply_ln1, apply_ln2, sim_compat=False, dbg=False):
    key = (kpt, apply_ln1, apply_ln2, sim_compat, dbg)
    if key not in _PROGRAM_CACHE:
        _PROGRAM_CACHE[key] = build_program(*key)
    return _PROGRAM_CACHE[key]


def _prep_inputs(x, mask, freqs_cos, freqs_sin, wq, wk, wv, wo, w1, w2,
                 ln1_w, ln1_b, ln2_w, ln2_b):
    """Build the 8 per-core input dicts.  Returns (kpt, in_maps)."""
    f32 = np.float32
    fp8 = ml_dtypes.float8_e4m3
    x = np.asarray(x, f32)
    mask = np.asarray(mask)
    cos = np.asarray(freqs_cos, f32)
    sin = np.asarray(freqs_sin, f32)
    B = x.shape[0]

    # interleaved +-, pre-scaled rope tables for all TB positions
    ci = np.empty((TB, HD), f32)
    ci[:, 0::2] = cos
    ci[:, 1::2] = cos
    si = np.empty((TB, HD), f32)
    si[:, 0::2] = -sin
    si[:, 1::2] = sin
    ci *= RWS
    si *= RWS

    idxs = [np.where(~mask[b])[0] for b in range(B)]
    kmax = max(1, max(len(ix) for ix in idxs))
    kpt = -(-kmax // P)          # ceil tiles
    kpt += kpt & 1               # even for DR pairing
    kpt = min(max(kpt, 2), 8)
    KP = kpt * P

    def tok_layout(t):
        return np.ascontiguousarray(
            t.reshape(-1, P, t.shape[-1] if t.ndim > 1 else 1)
            .transpose(1, 0, 2))

    def wlayout(w, kc):
        w = np.asarray(w, f32)
        return np.ascontiguousarray(
            (w * WS).reshape(kc, P, w.shape[1]).transpose(1, 0, 2)).astype(fp8)

    # w1a [P, FB, KC, P] bf16, fb-major: w1a[p, fb, c, t] = w1[c*128+p, fb*128+t]
    w1f = np.asarray(w1, f32).reshape(KC, P, FB, P)
    w1a = np.ascontiguousarray(w1f.transpose(1, 2, 0, 3)).astype(
        ml_dtypes.bfloat16)

    shared = {
        "wq": wlayout(wq, KC), "wk": wlayout(wk, KC), "wv": wlayout(wv, KC),
        "wo": wlayout(wo, KC),
        "w1a": w1a,
        "w2": wlayout(w2, FF // P),
        "ln1w": np.asarray(ln1_w, f32).reshape(1, D),
        "ln1b": np.asarray(ln1_b, f32).reshape(1, D),
        "ln2w": np.asarray(ln2_w, f32).reshape(1, D),
        "ln2b": np.asarray(ln2_b, f32).reshape(1, D),
    }

    in_maps = []
    for c in range(NCORES):
        b, half = divmod(c, 2)
        ix = idxs[b]
        npad = KP - len(ix)
        ixp = np.concatenate([ix, np.zeros(npad, np.int64)])
        mb = np.concatenate([np.zeros(len(ix), f32),
                             np.full(npad, MBIAS, f32)])
        lo = half * TL
        m = dict(shared)
        m["xq"] = np.ascontiguousarray(x[b, lo:lo + TL])
        m["xk"] = np.ascontiguousarray(x[b][ixp])
        m["mb"] = np.ascontiguousarray(tok_layout(mb)[:, :, 0])
        m["cosq"] = tok_layout(ci[lo:lo + TL])
        m["sinq"] = tok_layout(si[lo:lo + TL])
        m["cosk"] = tok_layout(ci[ixp])
        m["sink"] = tok_layout(si[ixp])
        in_maps.append(m)
    return kpt, in_maps


def kernel(x, mask, freqs_cos, freqs_sin, wq, wk, wv, wo, w1, w2,
           ln1_w, ln1_b, ln2_w, ln2_b, _trace=False, _sim=False, _dbg=False):
    from concourse.bass_utils import run_bass_kernel_spmd

    apply_ln1 = not (np.all(np.asarray(ln1_w) == 1.0)
                     and np.all(np.asarray(ln1_b) == 0.0))
    apply_ln2 = not (np.all(np.asarray(ln2_w) == 1.0)
                     and np.all(np.asarray(ln2_b) == 0.0))
    kpt, in_maps = _prep_inputs(x, mask, freqs_cos, freqs_sin, wq, wk, wv,
                                wo, w1, w2, ln1_w, ln1_b, ln2_w, ln2_b)
    nc = _get_program(kpt, apply_ln1, apply_ln2,
                      sim_compat=(_sim is not False and _sim is not None),
                      dbg=_dbg)

    if _sim is not False and _sim is not None:
        cidx = 0 if _sim is True else int(_sim)
        from concourse.bass_interp import CoreSim
        sim = CoreSim(nc, trace=False)
        for k, v in in_maps[cidx].items():
            sim.tensor(k)[:] = v
        sim.simulate(check_with_hw=False)
        b, half = divmod(cidx, 2)
        full = np.zeros((4, TB, D), np.float32)
        full[b, half * TL:(half + 1) * TL] = np.array(sim.tensor("out"))
        if _dbg:
            return full, sim
        return full

    res = run_bass_kernel_spmd(nc, in_maps, core_ids=list(range(NCORES)),
                               trace=_trace)
    full = np.empty((4, TB, D), np.float32)
    for c in range(NCORES):
        b, half = divmod(c, 2)
        full[b, half * TL:(half + 1) * TL] = res.results[c]["out"]
    if _trace:
        return full, res
    return full


# revision 38
# speedup vs baseline: 1.0187x; 1.0187x over previous
"""Trainium2 Bass kernel for a dense transformer block (nn_Block_3453153706485).

B=4, S=1024, D=1024, H=16 heads (hd=64), FF=4096, fp32 I/O.
Sharding: 8 cores; core c owns (batch b=c//2, token half c%2) -> 512 query
tokens.  Keys are mask-compressed: only unmasked key tokens (padded to a
multiple of 256) are projected / attended, since masked keys contribute
exp(-inf)=0 to softmax.

Per-core pipeline:
  A:  LN1 of the 4 local query tiles (xq) and kpt gathered key tiles (xk);
      h (bf16) -> PE-transpose -> hqT/hkT fp8 [D, tok].
  B:  q/k/v projections via fp8 DoubleRow matmuls against 32x-scaled fp8
      weights; RoPE (tables pre-scaled by 1/32) via one psum->bf16 eviction
      + two Pool multiplies + Pool add -> kr/qr bf16 -> PE-transpose ->
      kT/qT bf16.  v is evicted (psum->fp8, kept 32x scaled) into
      v1 [ktok, skc, head, 66] with col 64 = 32.0 (the 32-scaled softmax
      denominator row).
  C1: per key tile skc, per head pair j: scoresT = kT_h.T @ qT_h (bf16);
      exp(s/8 + mask_bias) -> pT fp8 [ktok, skc, head, q].
  C2: PV in [q, d] orientation: out[q, head, 0:66] += pT.T @ v1 (DR);
      col 64 = 32Z.  Normalize with a per-(q,head) reciprocal broadcast
      along the free dim, -> attn bf16 -> PE-transpose -> attnT fp8.
  D:  wo (DR) + residual via scalar_tensor_tensor (x 1/32 folded in);
      LN2 -> h2 bf16 -> transpose -> h2Tb bf16; fp8 split h2Th = fp8(h2Tb),
      h2Tl = fp8(8*(h2Tb - h2Th)) for the compensated fc1.
  E:  fc1 = h2Th@w1hi + h2Th@(w1lo/8) + h2Tl@(w1hi/8), three fp8 DR chains
      into one psum (32x scaled); w1 chunks streamed just-in-time from one
      interleaved DRAM tensor.  gelu applies the 1/32 descale -> g1 fp8;
      fc2 fp8 DR; final residual via scalar_tensor_tensor.
"""

from contextlib import ExitStack

import ml_dtypes
import numpy as np

import concourse.bass as bass
import concourse.tile as tile
from concourse import bacc, mybir
from concourse.masks import make_identity

F32 = mybir.dt.float32
BF16 = mybir.dt.bfloat16
F8 = mybir.dt.float8e4
AF = mybir.ActivationFunctionType
OP = mybir.AluOpType
DR = mybir.MatmulPerfMode.DoubleRow

P = 128
D = 1024
H = 16
HD = 64
FF = 4096
FB = FF // P       # 32 fc1 output blocks
TB = 1024          # tokens per batch
TL = 512           # local (query) tokens per core
QT = TL // P       # 4 query tiles
KC = D // P        # 8
NCORES = 8
EPS = 1e-5
WS = 32.0          # fp8 weight upscale
RWS = float(1.0 / WS)
MBIAS = -30.0      # exp bias for padding keys


def _swap_pairs(ap4):
    """View with the two elements of each innermost [step,2] pair swapped."""
    st = ap4.ap[-1][0]
    return bass.AP(
        tensor=ap4.tensor,
        offset=ap4.offset + st,
        ap=list(ap4.ap[:-1]) + [[-st, 2]],
    )


def build_program(kpt: int, apply_ln1: bool, apply_ln2: bool,
                  sim_compat: bool = False, dbg: bool = False):
    assert kpt % 2 == 0 and 2 <= kpt <= 8
    KP = kpt * P
    nc = bacc.Bacc("TRN2", target_bir_lowering=False, debug=False)

    xq_d = nc.dram_tensor("xq", [TL, D], F32, kind="ExternalInput").ap()
    xk_d = nc.dram_tensor("xk", [KP, D], F32, kind="ExternalInput").ap()
    mb_d = nc.dram_tensor("mb", [P, kpt], F32, kind="ExternalInput").ap()
    cosq_d = nc.dram_tensor("cosq", [P, QT, HD], F32, kind="ExternalInput").ap()
    sinq_d = nc.dram_tensor("sinq", [P, QT, HD], F32, kind="ExternalInput").ap()
    cosk_d = nc.dram_tensor("cosk", [P, kpt, HD], F32, kind="ExternalInput").ap()
    sink_d = nc.dram_tensor("sink", [P, kpt, HD], F32, kind="ExternalInput").ap()
    wq_d = nc.dram_tensor("wq", [P, KC, D], F8, kind="ExternalInput").ap()
    wk_d = nc.dram_tensor("wk", [P, KC, D], F8, kind="ExternalInput").ap()
    wv_d = nc.dram_tensor("wv", [P, KC, D], F8, kind="ExternalInput").ap()
    wo_d = nc.dram_tensor("wo", [P, KC, D], F8, kind="ExternalInput").ap()
    # fc1 weights, fb-major bf16 chunks for just-in-time streaming
    w1a_d = nc.dram_tensor("w1a", [P, FB, KC, P], BF16,
                           kind="ExternalInput").ap()
    w2_d = nc.dram_tensor("w2", [P, FF // P, D], F8, kind="ExternalInput").ap()
    ln_d = {nm: nc.dram_tensor(nm, [1, D], F32, kind="ExternalInput").ap()
            for nm in ("ln1w", "ln1b", "ln2w", "ln2b")}
    out_d = nc.dram_tensor("out", [TL, D], F32, kind="ExternalOutput").ap()
    dbg_d = {}
    if dbg:
        for nm, shape, dt in [
                ("d_hqT", [P, KC, TL], F8), ("d_hkT", [P, KC, KP], F8),
                ("d_kT", [P, KC, KP], BF16), ("d_qT", [P, KC, TL], BF16),
                ("d_v1", [P, kpt, H, 66], F8),
                ("d_at", [P, KC, TL], F8), ("d_xr", [P, QT, D], F32),
                ("d_g1", [P, FF // P, TL], F8), ("d_araw", [P, QT, H, 66], F32)]:
            dbg_d[nm] = nc.dram_tensor(nm, shape, dt,
                                       kind="ExternalOutput").ap()

    gelu_f = AF.Identity if sim_compat else AF.Gelu

    with tile.TileContext(nc) as tc:
        es0 = ExitStack()

        # ---- left pools (live to program end, released in reverse) ----
        cons = es0.enter_context(tc.tile_pool(name="cons", bufs=1))
        work = es0.enter_context(tc.tile_pool(name="work", bufs=2))
        pool_xq = tc.alloc_tile_pool(name="p_xq", bufs=1)
        xq_sb = pool_xq.tile([P, QT, D], F32, name="xq_sb")
        pool_kq = tc.alloc_tile_pool(name="p_kq", bufs=1)
        qT = pool_kq.tile([P, KC, TL], BF16, name="qT")
        kT = pool_kq.tile([P, KC, KP], BF16, name="kT")
        pool_v1 = tc.alloc_tile_pool(name="p_v1", bufs=1)
        v1 = pool_v1.tile([P, kpt, H, 66], F8, name="v1")

        # ---- right pools; stack order encodes release order ----
        pool_w2 = tc.alloc_tile_pool(name="p_w2", bufs=1, side="right")
        w2_sb = pool_w2.tile([P, FF // P, D], F8, name="w2_sb")
        pool_wD = tc.alloc_tile_pool(name="p_wD", bufs=1, side="right")
        wo_sb = pool_wD.tile([P, KC, D], F8, name="wo_sb")
        pool_pT = tc.alloc_tile_pool(name="p_pT", bufs=1, side="right")
        pool_hT = tc.alloc_tile_pool(name="p_hT", bufs=1, side="right")
        hqT = pool_hT.tile([P, KC, TL], F8, name="hqT")
        hkT = pool_hT.tile([P, KC, KP], F8, name="hkT")
        workB = tc.alloc_tile_pool(name="workB", bufs=2, side="right")
        pool_wB = tc.alloc_tile_pool(name="p_wB", bufs=1, side="right")
        wq_sb = pool_wB.tile([P, KC, D], F8, name="wq_sb")
        wk_sb = pool_wB.tile([P, KC, D], F8, name="wk_sb")
        wv_sb = pool_wB.tile([P, KC, D], F8, name="wv_sb")

        # ---------------- constants + weight DMAs ----------------
        identb = cons.tile([P, P], BF16, name="identb")
        make_identity(nc, identb)
        eps_t = cons.tile([P, 1], F32, name="eps_t")
        nc.vector.memset(eps_t[:], EPS)
        mbias = cons.tile([P, kpt], F32, name="mb_sb")
        nc.gpsimd.dma_start(mbias[:], mb_d)
        cosq = cons.tile([P, QT, HD], F32, name="cosq_sb")
        nc.gpsimd.dma_start(cosq[:], cosq_d)
        sinq = cons.tile([P, QT, HD], F32, name="sinq_sb")
        nc.gpsimd.dma_start(sinq[:], sinq_d)
        cosk = cons.tile([P, kpt, HD], F32, name="cosk_sb")
        nc.gpsimd.dma_start(cosk[:], cosk_d)
        sink = cons.tile([P, kpt, HD], F32, name="sink_sb")
        nc.gpsimd.dma_start(sink[:], sink_d)

        ln_bc = {}
        for nm, need in (("ln1w", apply_ln1), ("ln1b", apply_ln1),
                         ("ln2w", apply_ln2), ("ln2b", apply_ln2)):
            if need:
                t = cons.tile([P, D], F32, name=f"{nm}_bc")
                src = bass.AP(tensor=ln_d[nm].tensor, offset=ln_d[nm].offset,
                              ap=[[0, P], [1, D]])
                nc.gpsimd.dma_start(t[:], src)
                ln_bc[nm] = t

        # q/k/v weights on the gpsimd SWDGE queue (x tiles own the sync
        # queue; the ACT queue must stay clear for the LN sqrt/apply chain)
        nc.gpsimd.dma_start(wq_sb[:], wq_d)
        nc.gpsimd.dma_start(wk_sb[:], wk_d)
        nc.gpsimd.dma_start(wv_sb[:], wv_d)

        nc.vector.memset(v1[:, :, :, 64:65], WS)
        nc.vector.memset(v1[:, :, :, 65:66], 0.0)

        # psum pool for A/B/C1: pj f32 [P,D] (2 banks) x3, tpb bf16 x2
        psAB = tc.alloc_tile_pool(name="psAB", bufs=1, space="PSUM")

        def layernorm(src_ap, dst_ap, wkey, bkey, applied, eng="act"):
            stats = work.tile([P, 2, 6], F32, tag="stats", name="stats")
            nc.vector.bn_stats(stats[:, 0, :], src_ap[:, 0:512])
            nc.vector.bn_stats(stats[:, 1, :], src_ap[:, 512:1024])
            mv = work.tile([P, 2], F32, tag="mv", name="mv")
            nc.vector.bn_aggr(mv[:], stats[:])
            std = work.tile([P, 1], F32, tag="std", name="std")
            nc.scalar.activation(std[:], mv[:, 1:2], AF.Sqrt, bias=eps_t[:])
            rstd = work.tile([P, 1], F32, tag="rstd", name="rstd")
            nc.vector.reciprocal(rstd[:], std[:])
            if eng == "act":
                # (x - m)*rstd == Identity(rstd*x + (-m*rstd)) on ACT
                nmr = work.tile([P, 1], F32, tag="nmr", name="nmr")
                nc.vector.tensor_scalar(nmr[:], mv[:, 0:1], rstd[:], -1.0,
                                        OP.mult, OP.mult)
                nc.scalar.activation(dst_ap, src_ap, AF.Identity,
                                     scale=rstd[:], bias=nmr[:])
            else:
                nc.vector.tensor_scalar(dst_ap, src_ap, mv[:, 0:1], rstd[:],
                                        OP.subtract, OP.mult)
            if applied:
                nc.gpsimd.tensor_mul(dst_ap, dst_ap, ln_bc[wkey][:])
                nc.gpsimd.tensor_add(dst_ap, dst_ap, ln_bc[bkey][:])

        def transpose8(pool, h_ap, dst_ap, evict_eng):
            """h_ap [128, 1024] bf16 -> dst_ap [128, 8, 128] (cast on evict)."""
            pst = pool.tile([P, D], BF16, tag="tpb", bufs=2, name="pst")
            for c in range(KC):
                nc.tensor.transpose(pst[:, c * P:(c + 1) * P],
                                    h_ap[:, c * P:(c + 1) * P], identb[:])
            evict_eng(dst_ap, pst.rearrange("p (k t) -> p k t", k=KC))

        def proj_dr(ps_half, lhs3, w_sb, nh):
            for k2 in range(KC // 2):
                nc.tensor.matmul(
                    ps_half, lhsT=lhs3[:, 2 * k2:2 * k2 + 2, :],
                    rhs=w_sb[:, 2 * k2:2 * k2 + 2, nh * 512:(nh + 1) * 512],
                    start=(k2 == 0), stop=(k2 == KC // 2 - 1),
                    perf_mode=DR)

        def rope(ps, cos_sb, sin_sb, ti, kr_ap, add_eng):
            """psum [128,1024] f32 -> kr_ap bf16, rotated, 1/32 descale in
            the tables.  Two DVE psum reads, Pool add."""
            ps_h = ps.rearrange("p (h i) -> p h i", h=H)
            cos_b = cos_sb[:, ti, None, :].to_broadcast((P, H, HD))
            p1 = workB.tile([P, D], BF16, tag="p1", bufs=2, name=f"p1{ti}")
            nc.vector.tensor_tensor(p1.rearrange("p (h i) -> p h i", h=H),
                                    ps_h, cos_b, OP.mult)
            ps_sw = _swap_pairs(ps.rearrange("p (h i two) -> p h i two",
                                             h=H, two=2))
            sin_b = (sin_sb[:, ti, None, :].to_broadcast((P, H, HD))
                     .rearrange("p h (i two) -> p h i two", two=2))
            p2 = workB.tile([P, D], BF16, tag="p2", bufs=2, name=f"p2{ti}")
            nc.vector.tensor_tensor(
                p2.rearrange("p (h i two) -> p h i two", h=H, two=2),
                ps_sw, sin_b, OP.mult)
            add_eng(kr_ap, p1[:], p2[:])

        # prefetch all key tiles (scalar queue; the sync HWDGE carries xq)
        xks = []
        for ti in range(kpt):
            xk = workB.tile([P, D], F32, tag="xk", bufs=kpt, name=f"xk{ti}")
            nc.scalar.dma_start(xk[:], xk_d[ti * P:(ti + 1) * P, :])
            xks.append(xk)

        # ========== stage A+B: q group then k group, each stage-major ====
        # Group-local stage-major emission: deep pipelining within a group,
        # and q work is never queued behind k-tile DMA arrival.
        NT = QT + kpt

        def srcx(t):
            return xq_sb[:, t, :] if t < QT else xks[t - QT][:]

        def hT_slice(t):
            if t < QT:
                return hqT[:, :, t * P:(t + 1) * P]
            return hkT[:, :, (t - QT) * P:(t - QT + 1) * P]

        for t in range(QT):
            nc.sync.dma_start(xq_sb[:, t, :], xq_d[t * P:(t + 1) * P, :])
        # late-phase weights behind the x tiles on the sync queue
        nc.sync.dma_start(wo_sb[:], wo_d)
        nc.sync.dma_start(w2_sb[:], w2_d)

        rstds = [None] * NT

        def ln_stats(t):
            stats = work.tile([P, 2, 6], F32, tag="stA", bufs=3,
                              name=f"st{t}")
            nc.vector.bn_stats(stats[:, 0, :], srcx(t)[:, 0:512])
            nc.vector.bn_stats(stats[:, 1, :], srcx(t)[:, 512:1024])
            mv = work.tile([P, 2], F32, tag="mvA", bufs=3, name=f"mv{t}")
            nc.vector.bn_aggr(mv[:], stats[:])
            std = work.tile([P, 1], F32, tag="stdA", bufs=3, name=f"std{t}")
            nc.scalar.activation(std[:], mv[:, 1:2], AF.Sqrt, bias=eps_t[:])
            rs = work.tile([P, 2], F32, tag="rsA", bufs=NT, name=f"rs{t}")
            nc.vector.reciprocal(rs[:, 0:1], std[:])
            nc.vector.tensor_scalar(rs[:, 1:2], mv[:, 0:1], rs[:, 0:1], -1.0,
                                    OP.mult, OP.mult)
            rstds[t] = rs

        def ln_apply(t):
            rs = rstds[t]
            h = work.tile([P, D], BF16, tag="h", bufs=3, name=f"h{t}")
            nc.scalar.activation(h[:], srcx(t), AF.Identity,
                                 scale=rs[:, 0:1], bias=rs[:, 1:2])
            if apply_ln1:
                nc.gpsimd.tensor_mul(h[:], h[:], ln_bc["ln1w"][:])
                nc.gpsimd.tensor_add(h[:], h[:], ln_bc["ln1b"][:])
            transpose8(psAB, h[:], hT_slice(t), nc.scalar.copy)

        def proj_rope(t):
            isq = t < QT
            w_sb = wq_sb if isq else wk_sb
            ps = psAB.tile([P, D], F32, tag="pj", bufs=3, name=f"pj{t}")
            for nh in range(2):
                proj_dr(ps[:, nh * 512:(nh + 1) * 512], hT_slice(t),
                        w_sb, nh)
            kr = workB.tile([P, D], BF16, tag="kr", bufs=2, name=f"kr{t}")
            if isq:
                rope(ps, cosq, sinq, t, kr[:], nc.vector.tensor_add)
                transpose8(psAB, kr[:], qT[:, :, t * P:(t + 1) * P],
                           nc.scalar.copy)
            else:
                tk = t - QT
                rope(ps, cosk, sink, tk, kr[:], nc.gpsimd.tensor_add)
                transpose8(psAB, kr[:], kT[:, :, tk * P:(tk + 1) * P],
                           nc.scalar.copy)

        for t in range(NT):
            ln_stats(t)
        for t in range(NT):
            ln_apply(t)
        for t in range(NT):
            proj_rope(t)

        # v projections -> v1 (kept 32x scaled); evict on DVE so the ACT
        # queue is free to start the exp phase immediately
        for tk in range(kpt):
            psv = psAB.tile([P, D], F32, tag="pj", bufs=3, name=f"vps{tk}")
            for nh in range(2):
                proj_dr(psv[:, nh * 512:(nh + 1) * 512],
                        hkT[:, :, tk * P:(tk + 1) * P], wv_sb, nh)
            nc.vector.tensor_copy(v1[:, tk, :, 0:64],
                                  psv.rearrange("p (h d) -> p h d", h=H))

        if dbg:
            nc.sync.dma_start(dbg_d["d_hqT"], hqT[:])
            nc.sync.dma_start(dbg_d["d_hkT"], hkT[:])
            nc.sync.dma_start(dbg_d["d_kT"], kT[:])
            nc.sync.dma_start(dbg_d["d_qT"], qT[:])
            nc.sync.dma_start(dbg_d["d_v1"], v1[:])

        pool_wB.release()
        workB.release()
        pool_hT.release()

        # ========== stage C: per head pair: scores, exp, PV, normalize ====
        psAB.release()
        psC2 = tc.alloc_tile_pool(name="psC2", bufs=1, space="PSUM")
        pool_at = tc.alloc_tile_pool(name="p_at", bufs=1)
        attnT = pool_at.tile([P, KC, TL], F8, name="attnT")
        attn_sb = pool_at.tile([P, QT, H, HD], BF16, name="attn_sb")

        def pv_finish(j, pTj):
            pv = psC2.tile([P, QT, 2, P], F32, tag="pv", bufs=2,
                           name=f"pv{j}")
            for qb in range(QT):
                for hhi in range(2):
                    for sp in range(kpt // 2):
                        nc.tensor.matmul(
                            pv[:, qb, hhi, 0:66],
                            lhsT=pTj[:, 2 * sp:2 * sp + 2, hhi,
                                     qb * P:(qb + 1) * P],
                            rhs=v1[:, 2 * sp:2 * sp + 2, 2 * j + hhi, :],
                            start=(sp == 0), stop=(sp == kpt // 2 - 1),
                            perf_mode=DR)
            araw = work.tile([P, QT, 2, 66], F32, tag="araw", bufs=2,
                             name=f"araw{j}")
            nc.vector.tensor_copy(araw[:], pv[:, :, :, 0:66])
            if dbg:
                nc.sync.dma_start(dbg_d["d_araw"][:, :, 2 * j:2 * j + 2, :],
                                  araw[:])
            rz = work.tile([P, QT, 2], F32, tag="rz", bufs=2, name=f"rz{j}")
            nc.vector.reciprocal(rz[:], araw[:, :, :, 64])
            nc.gpsimd.tensor_mul(
                attn_sb[:, :, 2 * j:2 * j + 2, :], araw[:, :, :, 0:64],
                rz[:, :, :, None].to_broadcast((P, QT, 2, HD)))

        prev = None
        for j in range(H // 2):
            pTj = pool_pT.tile([P, kpt, 2, TL], F8, tag="pTr", bufs=2,
                               name=f"pT{j}")
            for skc in range(kpt):
                pss = psC2.tile([P, D], F32, tag="sc", bufs=2,
                                name=f"sc{j}_{skc}")
                for eo in range(2):
                    pb = 64 * eo
                    nc.tensor.matmul(
                        pss[:, eo * 512:(eo + 1) * 512],
                        lhsT=kT[pb:pb + 64, j, skc * P:(skc + 1) * P],
                        rhs=qT[pb:pb + 64, j, :],
                        start=True, stop=True)
                nc.scalar.activation(
                    pTj[:, skc, :, :], pss[:], AF.Exp,
                    scale=0.125, bias=mbias[:, skc:skc + 1])
            if prev is not None:
                pv_finish(*prev)
            prev = (j, pTj)
        pv_finish(*prev)

        psC2.release()
        psD = tc.alloc_tile_pool(name="psD", bufs=1, space="PSUM")
        pool_w1s = tc.alloc_tile_pool(name="p_w1s", bufs=1, side="right")

        def transpose8d(h_ap, dst_ap, evict_eng):
            transpose8(psD, h_ap, dst_ap, evict_eng)

        for qb in range(QT):
            transpose8d(attn_sb[:, qb, :, :].rearrange("p h d -> p (h d)"),
                        attnT[:, :, qb * P:(qb + 1) * P], nc.scalar.copy)

        # ========== stage D: wo + residual, LN2, h2T hi/lo ==========
        pool_res = tc.alloc_tile_pool(name="p_res", bufs=1)
        xres = pool_res.tile([P, QT, D], F32, name="xres")
        h2Tb = pool_res.tile([P, KC, TL], BF16, name="h2Tb")

        # stage-major: all wo+residuals, then all LN2 stats, then all applies
        for tc4 in range(QT):
            for nh in range(2):
                ps = psD.tile([P, 512], F32, tag="wo", bufs=3,
                              name=f"wops{tc4}_{nh}")
                for k2 in range(KC // 2):
                    nc.tensor.matmul(
                        ps[:], lhsT=attnT[:, 2 * k2:2 * k2 + 2,
                                          tc4 * P:(tc4 + 1) * P],
                        rhs=wo_sb[:, 2 * k2:2 * k2 + 2,
                                  nh * 512:(nh + 1) * 512],
                        start=(k2 == 0), stop=(k2 == KC // 2 - 1),
                        perf_mode=DR)
                nc.vector.scalar_tensor_tensor(
                    xres[:, tc4, nh * 512:(nh + 1) * 512], ps[:], RWS,
                    xq_sb[:, tc4, nh * 512:(nh + 1) * 512],
                    OP.mult, OP.add)
        rs2 = []
        for tc4 in range(QT):
            stats = work.tile([P, 2, 6], F32, tag="stA", bufs=3,
                              name=f"st2{tc4}")
            nc.vector.bn_stats(stats[:, 0, :], xres[:, tc4, 0:512])
            nc.vector.bn_stats(stats[:, 1, :], xres[:, tc4, 512:1024])
            mv = work.tile([P, 2], F32, tag="mvA", bufs=3, name=f"mv2{tc4}")
            nc.vector.bn_aggr(mv[:], stats[:])
            std = work.tile([P, 1], F32, tag="stdA", bufs=3,
                            name=f"std2{tc4}")
            nc.scalar.activation(std[:], mv[:, 1:2], AF.Sqrt, bias=eps_t[:])
            rs = work.tile([P, 2], F32, tag="rs2", bufs=QT, name=f"rs2{tc4}")
            nc.vector.reciprocal(rs[:, 0:1], std[:])
            nc.vector.tensor_scalar(rs[:, 1:2], mv[:, 0:1], rs[:, 0:1], -1.0,
                                    OP.mult, OP.mult)
            rs2.append(rs)
        for tc4 in range(QT):
            h2 = work.tile([P, D], BF16, tag="h", bufs=3, name=f"h2{tc4}")
            nc.scalar.activation(h2[:], xres[:, tc4, :], AF.Identity,
                                 scale=rs2[tc4][:, 0:1], bias=rs2[tc4][:, 1:2])
            if apply_ln2:
                nc.gpsimd.tensor_mul(h2[:], h2[:], ln_bc["ln2w"][:])
                nc.gpsimd.tensor_add(h2[:], h2[:], ln_bc["ln2b"][:])
            transpose8d(h2[:], h2Tb[:, :, tc4 * P:(tc4 + 1) * P],
                        nc.vector.tensor_copy)

        if dbg:
            nc.sync.dma_start(dbg_d["d_at"], attnT[:])
            nc.sync.dma_start(dbg_d["d_xr"], xres[:])

        psD.release()

        # ========== stage E: MLP ==========
        psE = tc.alloc_tile_pool(name="psE", bufs=1, space="PSUM")
        pool_g1 = tc.alloc_tile_pool(name="p_g1", bufs=1)
        g1 = pool_g1.tile([P, FF // P, TL], F8, name="g1")

        for g in range(FB // 2):
            w1s = pool_w1s.tile([P, 2, KC, P], BF16, tag="w1s", bufs=3,
                                name=f"w1s{g}")
            weng = nc.sync if g % 2 == 0 else nc.scalar
            weng.dma_start(w1s[:], w1a_d[:, 2 * g:2 * g + 2, :, :])
            ps = psE.tile([P, 2, 512], F32, tag="m1", bufs=2,
                          name=f"m1ps{g}")
            for i in range(2):
                for c in range(KC):
                    nc.tensor.matmul(
                        ps[:, i, :],
                        lhsT=w1s[:, i, c, :],
                        rhs=h2Tb[:, c, :],
                        start=(c == 0), stop=(c == KC - 1))
            nc.scalar.activation(
                g1[:, 2 * g:2 * g + 2, :].rearrange("p a b -> p (a b)"),
                ps.rearrange("p a b -> p (a b)"), gelu_f)

        for nh in range(2):
            for tc4 in range(QT):
                ps = psE.tile([P, 512], F32, tag="m2", bufs=2,
                              name=f"m2ps{nh}_{tc4}")
                for kp2 in range(FF // 256):
                    nc.tensor.matmul(
                        ps[:],
                        lhsT=g1[:, 2 * kp2:2 * kp2 + 2,
                                tc4 * P:(tc4 + 1) * P],
                        rhs=w2_sb[:, 2 * kp2:2 * kp2 + 2,
                                  nh * 512:(nh + 1) * 512],
                        start=(kp2 == 0), stop=(kp2 == FF // 256 - 1),
                        perf_mode=DR)
                ot = work.tile([P, 512], F32, tag="osb", name=f"ot{nh}_{tc4}")
                nc.vector.scalar_tensor_tensor(
                    ot[:], ps[:], RWS,
                    xres[:, tc4, nh * 512:(nh + 1) * 512], OP.mult, OP.add)
                oeng = nc.sync if tc4 % 2 == 0 else nc.scalar
                oeng.dma_start(out_d[tc4 * P:(tc4 + 1) * P,
                                     nh * 512:(nh + 1) * 512], ot[:])

        if dbg:
            nc.sync.dma_start(dbg_d["d_g1"], g1[:])

        pool_g1.release()
        pool_res.release()
        pool_at.release()
        pool_v1.release()
        pool_kq.release()
        pool_xq.release()
        pool_w1s.release()
        pool_pT.release()
        pool_wD.release()
        pool_w2.release()
        psE.release()
        es0.close()

    nc.compile()
    return nc


# ---------------------------------------------------------------------------
# Host side
# ---------------------------------------------------------------------------

_PROGRAM_CACHE = {}


def _get_program(kpt, apply_ln1, apply_ln2, sim_compat=False, dbg=False):
    key = (kpt, apply_ln1, apply_ln2, sim_compat, dbg)
    if key not in _PROGRAM_CACHE:
        _PROGRAM_CACHE[key] = build_program(*key)
    return _PROGRAM_CACHE[key]


def _prep_inputs(x, mask, freqs_cos, freqs_sin, wq, wk, wv, wo, w1, w2,
                 ln1_w, ln1_b, ln2_w, ln2_b):
    """Build the 8 per-core input dicts.  Returns (kpt, in_maps)."""
    f32 = np.float32
    fp8 = ml_dtypes.float8_e4m3
    x = np.asarray(x, f32)
    mask = np.asarray(mask)
    cos = np.asarray(freqs_cos, f32)
    sin = np.asarray(freqs_sin, f32)
    B = x.shape[0]

    # interleaved +-, pre-scaled rope tables for all TB positions
    ci = np.empty((TB, HD), f32)
    ci[:, 0::2] = cos
    ci[:, 1::2] = cos
    si = np.empty((TB, HD), f32)
    si[:, 0::2] = -sin
    si[:, 1::2] = sin
    ci *= RWS
    si *= RWS

    idxs = [np.where(~mask[b])[0] for b in range(B)]
    kmax = max(1, max(len(ix) for ix in idxs))
    kpt = -(-kmax // P)          # ceil tiles
    kpt += kpt & 1               # even for DR pairing
    kpt = min(max(kpt, 2), 8)
    KP = kpt * P

    def tok_layout(t):
        return np.ascontiguousarray(
            t.reshape(-1, P, t.shape[-1] if t.ndim > 1 else 1)
            .transpose(1, 0, 2))

    def wlayout(w, kc):
        w = np.asarray(w, f32)
        return np.ascontiguousarray(
            (w * WS).reshape(kc, P, w.shape[1]).transpose(1, 0, 2)).astype(fp8)

    # w1a [P, FB, KC, P] bf16, fb-major: w1a[p, fb, c, t] = w1[c*128+p, fb*128+t]
    w1f = np.asarray(w1, f32).reshape(KC, P, FB, P)
    w1a = np.ascontiguousarray(w1f.transpose(1, 2, 0, 3)).astype(
        ml_dtypes.bfloat16)

    shared = {
        "wq": wlayout(wq, KC), "wk": wlayout(wk, KC), "wv": wlayout(wv, KC),
        "wo": wlayout(wo, KC),
        "w1a": w1a,
        "w2": wlayout(w2, FF // P),
        "ln1w": np.asarray(ln1_w, f32).reshape(1, D),
        "ln1b": np.asarray(ln1_b, f32).reshape(1, D),
        "ln2w": np.asarray(ln2_w, f32).reshape(1, D),
        "ln2b": np.asarray(ln2_b, f32).reshape(1, D),
    }

    in_maps = []
    for c in range(NCORES):
        b, half = divmod(c, 2)
        ix = idxs[b]
        npad = KP - len(ix)
        ixp = np.concatenate([ix, np.zeros(npad, np.int64)])
        mb = np.concatenate([np.zeros(len(ix), f32),
                             np.full(npad, MBIAS, f32)])
        lo = half * TL
        m = dict(shared)
        m["xq"] = np.ascontiguousarray(x[b, lo:lo + TL])
        m["xk"] = np.ascontiguousarray(x[b][ixp])
        m["mb"] = np.ascontiguousarray(tok_layout(mb)[:, :, 0])
        m["cosq"] = tok_layout(ci[lo:lo + TL])
        m["sinq"] = tok_layout(si[lo:lo + TL])
        m["cosk"] = tok_layout(ci[ixp])
        m["sink"] = tok_layout(si[ixp])
        in_maps.append(m)
    return kpt, in_maps


def kernel(x, mask, freqs_cos, freqs_sin, wq, wk, wv, wo, w1, w2,
           ln1_w, ln1_b, ln2_w, ln2_b, _trace=False, _sim=False, _dbg=False):
    from concourse.bass_utils import run_bass_kernel_spmd

    apply_ln1 = not (np.all(np.asarray(ln1_w) == 1.0)
                     and np.all(np.asarray(ln1_b) == 0.0))
    apply_ln2 = not (np.all(np.asarray(ln2_w) == 1.0)
                     and np.all(np.asarray(ln2_b) == 0.0))
    kpt, in_maps = _prep_inputs(x, mask, freqs_cos, freqs_sin, wq, wk, wv,
                                wo, w1, w2, ln1_w, ln1_b, ln2_w, ln2_b)
    nc = _get_program(kpt, apply_ln1, apply_ln2,
                      sim_compat=(_sim is not False and _sim is not None),
                      dbg=_dbg)

    if _sim is not False and _sim is not None:
        cidx = 0 if _sim is True else int(_sim)
        from concourse.bass_interp import CoreSim
        sim = CoreSim(nc, trace=False)
        for k, v in in_maps[cidx].items():
            sim.tensor(k)[:] = v
        sim.simulate(check_with_hw=False)
        b, half = divmod(cidx, 2)
        full = np.zeros((4, TB, D), np.float32)
        full[b, half * TL:(half + 1) * TL] = np.array(sim.tensor("out"))
        if _dbg:
            return full, sim
        return full

    res = run_bass_kernel_spmd(nc, in_maps, core_ids=list(range(NCORES)),
                               trace=_trace)
    full = np.empty((4, TB, D), np.float32)
    for c in range(NCORES):
        b, half = divmod(c, 2)
        full[b, half * TL:(half + 1) * TL] = res.results[c]["out"]
    if _trace:
        return full, res
    return full


# revision 40
# speedup vs baseline: 1.0549x; 1.0355x over previous
"""Trainium2 Bass kernel for a dense transformer block (nn_Block_3453153706485).

B=4, S=1024, D=1024, H=16 heads (hd=64), FF=4096, fp32 I/O.
Sharding: 8 cores; core c owns (batch b=c//2, token half c%2) -> 512 query
tokens.  Keys are mask-compressed: only unmasked key tokens (padded to a
multiple of 256) are projected / attended, since masked keys contribute
exp(-inf)=0 to softmax.

Per-core pipeline:
  A:  LN1 of the 4 local query tiles (xq) and kpt gathered key tiles (xk);
      h (bf16) -> PE-transpose -> hqT/hkT fp8 [D, tok].
  B:  q/k/v projections via fp8 DoubleRow matmuls against 32x-scaled fp8
      weights; RoPE (tables pre-scaled by 1/32) via one psum->bf16 eviction
      + two Pool multiplies + Pool add -> kr/qr bf16 -> PE-transpose ->
      kT/qT bf16.  v is evicted (psum->fp8, kept 32x scaled) into
      v1 [ktok, skc, head, 66] with col 64 = 32.0 (the 32-scaled softmax
      denominator row).
  C1: per key tile skc, per head pair j: scoresT = kT_h.T @ qT_h (bf16);
      exp(s/8 + mask_bias) -> pT fp8 [ktok, skc, head, q].
  C2: PV in [q, d] orientation: out[q, head, 0:66] += pT.T @ v1 (DR);
      col 64 = 32Z.  Normalize with a per-(q,head) reciprocal broadcast
      along the free dim, -> attn bf16 -> PE-transpose -> attnT fp8.
  D:  wo (DR) + residual via scalar_tensor_tensor (x 1/32 folded in);
      LN2 -> h2 bf16 -> transpose -> h2Tb bf16; fp8 split h2Th = fp8(h2Tb),
      h2Tl = fp8(8*(h2Tb - h2Th)) for the compensated fc1.
  E:  fc1 = h2Th@w1hi + h2Th@(w1lo/8) + h2Tl@(w1hi/8), three fp8 DR chains
      into one psum (32x scaled); w1 chunks streamed just-in-time from one
      interleaved DRAM tensor.  gelu applies the 1/32 descale -> g1 fp8;
      fc2 fp8 DR; final residual via scalar_tensor_tensor.
"""

from contextlib import ExitStack

import ml_dtypes
import numpy as np

import concourse.bass as bass
import concourse.tile as tile
from concourse import bacc, mybir
from concourse.masks import make_identity

F32 = mybir.dt.float32
BF16 = mybir.dt.bfloat16
F8 = mybir.dt.float8e4
AF = mybir.ActivationFunctionType
OP = mybir.AluOpType
DR = mybir.MatmulPerfMode.DoubleRow

P = 128
D = 1024
H = 16
HD = 64
FF = 4096
FB = FF // P       # 32 fc1 output blocks
TB = 1024          # tokens per batch
TL = 512           # local (query) tokens per core
QT = TL // P       # 4 query tiles
KC = D // P        # 8
NCORES = 8
EPS = 1e-5
WS = 32.0          # fp8 weight upscale
RWS = float(1.0 / WS)
MBIAS = -30.0      # exp bias for padding keys


def _swap_pairs(ap4):
    """View with the two elements of each innermost [step,2] pair swapped."""
    st = ap4.ap[-1][0]
    return bass.AP(
        tensor=ap4.tensor,
        offset=ap4.offset + st,
        ap=list(ap4.ap[:-1]) + [[-st, 2]],
    )


def build_program(kpt: int, apply_ln1: bool, apply_ln2: bool,
                  sim_compat: bool = False, dbg: bool = False):
    assert kpt % 2 == 0 and 2 <= kpt <= 8
    KP = kpt * P
    nc = bacc.Bacc("TRN2", target_bir_lowering=False, debug=False)

    xq_d = nc.dram_tensor("xq", [TL, D], F32, kind="ExternalInput").ap()
    xk_d = nc.dram_tensor("xk", [KP, D], F32, kind="ExternalInput").ap()
    mb_d = nc.dram_tensor("mb", [P, kpt], F32, kind="ExternalInput").ap()
    cosq_d = nc.dram_tensor("cosq", [P, QT, HD], F32, kind="ExternalInput").ap()
    sinq_d = nc.dram_tensor("sinq", [P, QT, HD], F32, kind="ExternalInput").ap()
    cosk_d = nc.dram_tensor("cosk", [P, kpt, HD], F32, kind="ExternalInput").ap()
    sink_d = nc.dram_tensor("sink", [P, kpt, HD], F32, kind="ExternalInput").ap()
    wq_d = nc.dram_tensor("wq", [P, KC, D], F8, kind="ExternalInput").ap()
    wk_d = nc.dram_tensor("wk", [P, KC, D], F8, kind="ExternalInput").ap()
    wv_d = nc.dram_tensor("wv", [P, KC, D], F8, kind="ExternalInput").ap()
    wo_d = nc.dram_tensor("wo", [P, KC, D], F8, kind="ExternalInput").ap()
    # fc1 weights, fb-major bf16 chunks for just-in-time streaming
    w1a_d = nc.dram_tensor("w1a", [P, FB, KC, P], BF16,
                           kind="ExternalInput").ap()
    w2_d = nc.dram_tensor("w2", [P, FF // P, D], F8, kind="ExternalInput").ap()
    ln_d = {nm: nc.dram_tensor(nm, [1, D], F32, kind="ExternalInput").ap()
            for nm in ("ln1w", "ln1b", "ln2w", "ln2b")}
    out_d = nc.dram_tensor("out", [TL, D], F32, kind="ExternalOutput").ap()
    dbg_d = {}
    if dbg:
        for nm, shape, dt in [
                ("d_hqT", [P, KC, TL], F8), ("d_hkT", [P, KC, KP], F8),
                ("d_kT", [P, KC, KP], BF16), ("d_qT", [P, KC, TL], BF16),
                ("d_v1", [P, kpt, H, 66], F8),
                ("d_at", [P, KC, TL], F8), ("d_xr", [P, QT, D], F32),
                ("d_g1", [P, FF // P, TL], F8), ("d_araw", [P, QT, H, 66], F32)]:
            dbg_d[nm] = nc.dram_tensor(nm, shape, dt,
                                       kind="ExternalOutput").ap()

    gelu_f = AF.Identity if sim_compat else AF.Gelu

    with tile.TileContext(nc) as tc:
        es0 = ExitStack()

        # ---- left pools (live to program end, released in reverse) ----
        cons = es0.enter_context(tc.tile_pool(name="cons", bufs=1))
        work = es0.enter_context(tc.tile_pool(name="work", bufs=2))
        pool_xq = tc.alloc_tile_pool(name="p_xq", bufs=1)
        xq_sb = pool_xq.tile([P, QT, D], F32, name="xq_sb")
        pool_kq = tc.alloc_tile_pool(name="p_kq", bufs=1)
        qT = pool_kq.tile([P, KC, TL], BF16, name="qT")
        kT = pool_kq.tile([P, KC, KP], BF16, name="kT")
        pool_v1 = tc.alloc_tile_pool(name="p_v1", bufs=1)
        v1 = pool_v1.tile([P, kpt, H, 66], F8, name="v1")

        # ---- right pools; stack order encodes release order ----
        pool_w2 = tc.alloc_tile_pool(name="p_w2", bufs=1, side="right")
        w2_sb = pool_w2.tile([P, FF // P, D], F8, name="w2_sb")
        pool_wD = tc.alloc_tile_pool(name="p_wD", bufs=1, side="right")
        wo_sb = pool_wD.tile([P, KC, D], F8, name="wo_sb")
        pool_pT = tc.alloc_tile_pool(name="p_pT", bufs=1, side="right")
        pool_hT = tc.alloc_tile_pool(name="p_hT", bufs=1, side="right")
        hqT = pool_hT.tile([P, KC, TL], F8, name="hqT")
        hkT = pool_hT.tile([P, KC, KP], F8, name="hkT")
        workB = tc.alloc_tile_pool(name="workB", bufs=2, side="right")
        pool_wB = tc.alloc_tile_pool(name="p_wB", bufs=1, side="right")
        wq_sb = pool_wB.tile([P, KC, D], F8, name="wq_sb")
        wk_sb = pool_wB.tile([P, KC, D], F8, name="wk_sb")
        wv_sb = pool_wB.tile([P, KC, D], F8, name="wv_sb")

        # ---------------- constants + weight DMAs ----------------
        identb = cons.tile([P, P], BF16, name="identb")
        make_identity(nc, identb)
        eps_t = cons.tile([P, 1], F32, name="eps_t")
        nc.vector.memset(eps_t[:], EPS)
        mbias = cons.tile([P, kpt], F32, name="mb_sb")
        nc.gpsimd.dma_start(mbias[:], mb_d)
        cosq = cons.tile([P, QT, HD], F32, name="cosq_sb")
        nc.gpsimd.dma_start(cosq[:], cosq_d)
        sinq = cons.tile([P, QT, HD], F32, name="sinq_sb")
        nc.gpsimd.dma_start(sinq[:], sinq_d)
        cosk = cons.tile([P, kpt, HD], F32, name="cosk_sb")
        nc.gpsimd.dma_start(cosk[:], cosk_d)
        sink = cons.tile([P, kpt, HD], F32, name="sink_sb")
        nc.gpsimd.dma_start(sink[:], sink_d)

        ln_bc = {}
        for nm, need in (("ln1w", apply_ln1), ("ln1b", apply_ln1),
                         ("ln2w", apply_ln2), ("ln2b", apply_ln2)):
            if need:
                t = cons.tile([P, D], F32, name=f"{nm}_bc")
                src = bass.AP(tensor=ln_d[nm].tensor, offset=ln_d[nm].offset,
                              ap=[[0, P], [1, D]])
                nc.gpsimd.dma_start(t[:], src)
                ln_bc[nm] = t

        # q/k/v weights on the gpsimd SWDGE queue (x tiles own the sync
        # queue; the ACT queue must stay clear for the LN sqrt/apply chain)
        nc.gpsimd.dma_start(wq_sb[:], wq_d)
        nc.gpsimd.dma_start(wk_sb[:], wk_d)
        nc.gpsimd.dma_start(wv_sb[:], wv_d)

        nc.vector.memset(v1[:, :, :, 64:65], WS)
        nc.vector.memset(v1[:, :, :, 65:66], 0.0)

        # psum pool for A/B/C1: pj f32 [P,D] (2 banks) x3, tpb bf16 x2
        psAB = tc.alloc_tile_pool(name="psAB", bufs=1, space="PSUM")

        def layernorm(src_ap, dst_ap, wkey, bkey, applied, eng="act"):
            stats = work.tile([P, 2, 6], F32, tag="stats", name="stats")
            nc.vector.bn_stats(stats[:, 0, :], src_ap[:, 0:512])
            nc.vector.bn_stats(stats[:, 1, :], src_ap[:, 512:1024])
            mv = work.tile([P, 2], F32, tag="mv", name="mv")
            nc.vector.bn_aggr(mv[:], stats[:])
            std = work.tile([P, 1], F32, tag="std", name="std")
            nc.scalar.activation(std[:], mv[:, 1:2], AF.Sqrt, bias=eps_t[:])
            rstd = work.tile([P, 1], F32, tag="rstd", name="rstd")
            nc.vector.reciprocal(rstd[:], std[:])
            if eng == "act":
                # (x - m)*rstd == Identity(rstd*x + (-m*rstd)) on ACT
                nmr = work.tile([P, 1], F32, tag="nmr", name="nmr")
                nc.vector.tensor_scalar(nmr[:], mv[:, 0:1], rstd[:], -1.0,
                                        OP.mult, OP.mult)
                nc.scalar.activation(dst_ap, src_ap, AF.Identity,
                                     scale=rstd[:], bias=nmr[:])
            else:
                nc.vector.tensor_scalar(dst_ap, src_ap, mv[:, 0:1], rstd[:],
                                        OP.subtract, OP.mult)
            if applied:
                nc.gpsimd.tensor_mul(dst_ap, dst_ap, ln_bc[wkey][:])
                nc.gpsimd.tensor_add(dst_ap, dst_ap, ln_bc[bkey][:])

        def transpose8(pool, h_ap, dst_ap, evict_eng):
            """h_ap [128, 1024] bf16 -> dst_ap [128, 8, 128] (cast on evict)."""
            pst = pool.tile([P, D], BF16, tag="tpb", bufs=2, name="pst")
            for c in range(KC):
                nc.tensor.transpose(pst[:, c * P:(c + 1) * P],
                                    h_ap[:, c * P:(c + 1) * P], identb[:])
            evict_eng(dst_ap, pst.rearrange("p (k t) -> p k t", k=KC))

        def proj_dr(ps_half, lhs3, w_sb, nh):
            for k2 in range(KC // 2):
                nc.tensor.matmul(
                    ps_half, lhsT=lhs3[:, 2 * k2:2 * k2 + 2, :],
                    rhs=w_sb[:, 2 * k2:2 * k2 + 2, nh * 512:(nh + 1) * 512],
                    start=(k2 == 0), stop=(k2 == KC // 2 - 1),
                    perf_mode=DR)

        def rope(ps, cos_sb, sin_sb, ti, kr_ap, add_eng):
            """psum [128,1024] f32 -> kr_ap bf16, rotated, 1/32 descale in
            the tables.  Two DVE psum reads, Pool add."""
            ps_h = ps.rearrange("p (h i) -> p h i", h=H)
            cos_b = cos_sb[:, ti, None, :].to_broadcast((P, H, HD))
            p1 = workB.tile([P, D], BF16, tag="p1", bufs=2, name=f"p1{ti}")
            nc.vector.tensor_tensor(p1.rearrange("p (h i) -> p h i", h=H),
                                    ps_h, cos_b, OP.mult)
            ps_sw = _swap_pairs(ps.rearrange("p (h i two) -> p h i two",
                                             h=H, two=2))
            sin_b = (sin_sb[:, ti, None, :].to_broadcast((P, H, HD))
                     .rearrange("p h (i two) -> p h i two", two=2))
            p2 = workB.tile([P, D], BF16, tag="p2", bufs=2, name=f"p2{ti}")
            nc.vector.tensor_tensor(
                p2.rearrange("p (h i two) -> p h i two", h=H, two=2),
                ps_sw, sin_b, OP.mult)
            add_eng(kr_ap, p1[:], p2[:])

        # prefetch all key tiles (scalar queue; the sync HWDGE carries xq)
        xks = []
        for ti in range(kpt):
            xk = workB.tile([P, D], F32, tag="xk", bufs=kpt, name=f"xk{ti}")
            nc.scalar.dma_start(xk[:], xk_d[ti * P:(ti + 1) * P, :])
            xks.append(xk)

        # ========== stage A+B: q group then k group, each stage-major ====
        # Group-local stage-major emission: deep pipelining within a group,
        # and q work is never queued behind k-tile DMA arrival.
        NT = QT + kpt

        def srcx(t):
            return xq_sb[:, t, :] if t < QT else xks[t - QT][:]

        def hT_slice(t):
            if t < QT:
                return hqT[:, :, t * P:(t + 1) * P]
            return hkT[:, :, (t - QT) * P:(t - QT + 1) * P]

        for t in range(QT):
            nc.sync.dma_start(xq_sb[:, t, :], xq_d[t * P:(t + 1) * P, :])
        # late-phase weights behind the x tiles on the sync queue
        nc.sync.dma_start(wo_sb[:], wo_d)
        nc.sync.dma_start(w2_sb[:], w2_d)

        rstds = [None] * NT

        def ln_stats(t):
            stats = work.tile([P, 2, 6], F32, tag="stA", bufs=3,
                              name=f"st{t}")
            nc.vector.bn_stats(stats[:, 0, :], srcx(t)[:, 0:512])
            nc.vector.bn_stats(stats[:, 1, :], srcx(t)[:, 512:1024])
            mv = work.tile([P, 2], F32, tag="mvA", bufs=3, name=f"mv{t}")
            nc.vector.bn_aggr(mv[:], stats[:])
            std = work.tile([P, 1], F32, tag="stdA", bufs=3, name=f"std{t}")
            nc.scalar.activation(std[:], mv[:, 1:2], AF.Sqrt, bias=eps_t[:])
            rs = work.tile([P, 2], F32, tag="rsA", bufs=NT, name=f"rs{t}")
            nc.vector.reciprocal(rs[:, 0:1], std[:])
            nc.vector.tensor_scalar(rs[:, 1:2], mv[:, 0:1], rs[:, 0:1], -1.0,
                                    OP.mult, OP.mult)
            rstds[t] = rs

        def ln_apply(t):
            rs = rstds[t]
            h = work.tile([P, D], BF16, tag="h", bufs=3, name=f"h{t}")
            nc.scalar.activation(h[:], srcx(t), AF.Identity,
                                 scale=rs[:, 0:1], bias=rs[:, 1:2])
            if apply_ln1:
                nc.gpsimd.tensor_mul(h[:], h[:], ln_bc["ln1w"][:])
                nc.gpsimd.tensor_add(h[:], h[:], ln_bc["ln1b"][:])
            transpose8(psAB, h[:], hT_slice(t), nc.scalar.copy)

        def proj_rope(t):
            isq = t < QT
            w_sb = wq_sb if isq else wk_sb
            ps = psAB.tile([P, D], F32, tag="pj", bufs=3, name=f"pj{t}")
            for nh in range(2):
                proj_dr(ps[:, nh * 512:(nh + 1) * 512], hT_slice(t),
                        w_sb, nh)
            kr = workB.tile([P, D], BF16, tag="kr", bufs=2, name=f"kr{t}")
            if isq:
                rope(ps, cosq, sinq, t, kr[:], nc.vector.tensor_add)
                transpose8(psAB, kr[:], qT[:, :, t * P:(t + 1) * P],
                           nc.scalar.copy)
            else:
                tk = t - QT
                rope(ps, cosk, sink, tk, kr[:], nc.gpsimd.tensor_add)
                transpose8(psAB, kr[:], kT[:, :, tk * P:(tk + 1) * P],
                           nc.scalar.copy)

        for t in range(NT):
            ln_stats(t)
        for t in range(NT):
            ln_apply(t)
        for t in range(NT):
            proj_rope(t)

        # v projections -> v1 (kept 32x scaled); evict on DVE so the ACT
        # queue is free to start the exp phase immediately
        for tk in range(kpt):
            psv = psAB.tile([P, D], F32, tag="pj", bufs=3, name=f"vps{tk}")
            for nh in range(2):
                proj_dr(psv[:, nh * 512:(nh + 1) * 512],
                        hkT[:, :, tk * P:(tk + 1) * P], wv_sb, nh)
            nc.vector.tensor_copy(v1[:, tk, :, 0:64],
                                  psv.rearrange("p (h d) -> p h d", h=H))

        if dbg:
            nc.sync.dma_start(dbg_d["d_hqT"], hqT[:])
            nc.sync.dma_start(dbg_d["d_hkT"], hkT[:])
            nc.sync.dma_start(dbg_d["d_kT"], kT[:])
            nc.sync.dma_start(dbg_d["d_qT"], qT[:])
            nc.sync.dma_start(dbg_d["d_v1"], v1[:])

        pool_wB.release()
        workB.release()
        pool_hT.release()

        # ========== stage C: per head pair: scores, exp, PV, normalize ====
        psAB.release()
        psC2 = tc.alloc_tile_pool(name="psC2", bufs=1, space="PSUM")
        pool_at = tc.alloc_tile_pool(name="p_at", bufs=1)
        attnT = pool_at.tile([P, KC, TL], F8, name="attnT")
        attn_sb = pool_at.tile([P, QT, H, HD], BF16, name="attn_sb")

        def pv_finish(j, pTj):
            pv = psC2.tile([P, QT, 2, P], F32, tag="pv", bufs=2,
                           name=f"pv{j}")
            for qb in range(QT):
                for hhi in range(2):
                    for sp in range(kpt // 2):
                        nc.tensor.matmul(
                            pv[:, qb, hhi, 0:66],
                            lhsT=pTj[:, 2 * sp:2 * sp + 2, hhi,
                                     qb * P:(qb + 1) * P],
                            rhs=v1[:, 2 * sp:2 * sp + 2, 2 * j + hhi, :],
                            start=(sp == 0), stop=(sp == kpt // 2 - 1),
                            perf_mode=DR)
            araw = work.tile([P, QT, 2, 66], F32, tag="araw", bufs=2,
                             name=f"araw{j}")
            nc.vector.tensor_copy(araw[:], pv[:, :, :, 0:66])
            if dbg:
                nc.sync.dma_start(dbg_d["d_araw"][:, :, 2 * j:2 * j + 2, :],
                                  araw[:])
            rz = work.tile([P, QT, 2], F32, tag="rz", bufs=2, name=f"rz{j}")
            nc.vector.reciprocal(rz[:], araw[:, :, :, 64])
            nc.gpsimd.tensor_mul(
                attn_sb[:, :, 2 * j:2 * j + 2, :], araw[:, :, :, 0:64],
                rz[:, :, :, None].to_broadcast((P, QT, 2, HD)))

        prev = None
        for j in range(H // 2):
            pTj = pool_pT.tile([P, kpt, 2, TL], F8, tag="pTr", bufs=2,
                               name=f"pT{j}")
            for skc in range(kpt):
                pss = psC2.tile([P, D], F32, tag="sc", bufs=2,
                                name=f"sc{j}_{skc}")
                for eo in range(2):
                    pb = 64 * eo
                    nc.tensor.matmul(
                        pss[:, eo * 512:(eo + 1) * 512],
                        lhsT=kT[pb:pb + 64, j, skc * P:(skc + 1) * P],
                        rhs=qT[pb:pb + 64, j, :],
                        start=True, stop=True)
                nc.scalar.activation(
                    pTj[:, skc, :, :], pss[:], AF.Exp,
                    scale=0.125, bias=mbias[:, skc:skc + 1])
            if prev is not None:
                pv_finish(*prev)
            prev = (j, pTj)
        pv_finish(*prev)

        psC2.release()
        psD = tc.alloc_tile_pool(name="psD", bufs=1, space="PSUM")
        pool_w1s = tc.alloc_tile_pool(name="p_w1s", bufs=1, side="right")

        def transpose8d(h_ap, dst_ap, evict_eng):
            transpose8(psD, h_ap, dst_ap, evict_eng)

        for qb in range(QT):
            transpose8d(attn_sb[:, qb, :, :].rearrange("p h d -> p (h d)"),
                        attnT[:, :, qb * P:(qb + 1) * P], nc.scalar.copy)

        # ========== stage D: wo + residual, LN2, h2T hi/lo ==========
        pool_res = tc.alloc_tile_pool(name="p_res", bufs=1)
        xres = pool_res.tile([P, QT, D], F32, name="xres")
        h2Tb = pool_res.tile([P, KC, TL], BF16, name="h2Tb")

        # stage-major: all wo+residuals, then all LN2 stats, then all applies
        for tc4 in range(QT):
            for nh in range(2):
                ps = psD.tile([P, 512], F32, tag="wo", bufs=3,
                              name=f"wops{tc4}_{nh}")
                for k2 in range(KC // 2):
                    nc.tensor.matmul(
                        ps[:], lhsT=attnT[:, 2 * k2:2 * k2 + 2,
                                          tc4 * P:(tc4 + 1) * P],
                        rhs=wo_sb[:, 2 * k2:2 * k2 + 2,
                                  nh * 512:(nh + 1) * 512],
                        start=(k2 == 0), stop=(k2 == KC // 2 - 1),
                        perf_mode=DR)
                nc.vector.scalar_tensor_tensor(
                    xres[:, tc4, nh * 512:(nh + 1) * 512], ps[:], RWS,
                    xq_sb[:, tc4, nh * 512:(nh + 1) * 512],
                    OP.mult, OP.add)
        rs2 = []
        for tc4 in range(QT):
            stats = work.tile([P, 2, 6], F32, tag="stA", bufs=3,
                              name=f"st2{tc4}")
            nc.vector.bn_stats(stats[:, 0, :], xres[:, tc4, 0:512])
            nc.vector.bn_stats(stats[:, 1, :], xres[:, tc4, 512:1024])
            mv = work.tile([P, 2], F32, tag="mvA", bufs=3, name=f"mv2{tc4}")
            nc.vector.bn_aggr(mv[:], stats[:])
            std = work.tile([P, 1], F32, tag="stdA", bufs=3,
                            name=f"std2{tc4}")
            nc.scalar.activation(std[:], mv[:, 1:2], AF.Sqrt, bias=eps_t[:])
            rs = work.tile([P, 2], F32, tag="rs2", bufs=QT, name=f"rs2{tc4}")
            nc.vector.reciprocal(rs[:, 0:1], std[:])
            nc.vector.tensor_scalar(rs[:, 1:2], mv[:, 0:1], rs[:, 0:1], -1.0,
                                    OP.mult, OP.mult)
            rs2.append(rs)
        for tc4 in range(QT):
            h2 = work.tile([P, D], BF16, tag="h", bufs=3, name=f"h2{tc4}")
            nc.scalar.activation(h2[:], xres[:, tc4, :], AF.Identity,
                                 scale=rs2[tc4][:, 0:1], bias=rs2[tc4][:, 1:2])
            if apply_ln2:
                nc.gpsimd.tensor_mul(h2[:], h2[:], ln_bc["ln2w"][:])
                nc.gpsimd.tensor_add(h2[:], h2[:], ln_bc["ln2b"][:])
            transpose8d(h2[:], h2Tb[:, :, tc4 * P:(tc4 + 1) * P],
                        nc.vector.tensor_copy)

        if dbg:
            nc.sync.dma_start(dbg_d["d_at"], attnT[:])
            nc.sync.dma_start(dbg_d["d_xr"], xres[:])

        psD.release()

        # ========== stage E: MLP ==========
        psE = tc.alloc_tile_pool(name="psE", bufs=1, space="PSUM")
        pool_g1 = tc.alloc_tile_pool(name="p_g1", bufs=1)
        g1 = pool_g1.tile([P, FF // P, TL], F8, name="g1")

        # fc1/fc2 split by token halves: fc1 on tokens [0:256) starts as
        # soon as the first two h2T tiles exist; fc2 for those tokens then
        # overlaps fc1's second half.  w1 is streamed twice (cheap; the DMA
        # queues are idle during E).
        def fc2_tok(tc4):
            for nh in range(2):
                ps = psE.tile([P, 512], F32, tag="m2", bufs=2,
                              name=f"m2ps{nh}_{tc4}")
                for kp2 in range(FF // 256):
                    nc.tensor.matmul(
                        ps[:],
                        lhsT=g1[:, 2 * kp2:2 * kp2 + 2,
                                tc4 * P:(tc4 + 1) * P],
                        rhs=w2_sb[:, 2 * kp2:2 * kp2 + 2,
                                  nh * 512:(nh + 1) * 512],
                        start=(kp2 == 0), stop=(kp2 == FF // 256 - 1),
                        perf_mode=DR)
                ot = work.tile([P, 512], F32, tag="osb", name=f"ot{nh}_{tc4}")
                nc.vector.scalar_tensor_tensor(
                    ot[:], ps[:], RWS,
                    xres[:, tc4, nh * 512:(nh + 1) * 512], OP.mult, OP.add)
                oeng = nc.sync if nh % 2 == 0 else nc.scalar
                oeng.dma_start(out_d[tc4 * P:(tc4 + 1) * P,
                                     nh * 512:(nh + 1) * 512], ot[:])

        for g in range(FB // 2):
            w1s = pool_w1s.tile([P, 2, KC, P], BF16, tag="w1s", bufs=3,
                                name=f"w1s{g}")
            weng = nc.sync if g % 2 == 0 else nc.scalar
            weng.dma_start(w1s[:], w1a_d[:, 2 * g:2 * g + 2, :, :])
            ps = psE.tile([P, 2, 512], F32, tag="m1", bufs=2,
                          name=f"m1ps{g}")
            for i in range(2):
                for c in range(KC):
                    nc.tensor.matmul(
                        ps[:, i, :],
                        lhsT=w1s[:, i, c, :],
                        rhs=h2Tb[:, c, :],
                        start=(c == 0), stop=(c == KC - 1))
            nc.scalar.activation(
                g1[:, 2 * g:2 * g + 2, :].rearrange("p a b -> p (a b)"),
                ps.rearrange("p a b -> p (a b)"), gelu_f)
        for tc4 in range(QT):
            fc2_tok(tc4)

        if dbg:
            nc.sync.dma_start(dbg_d["d_g1"], g1[:])

        pool_g1.release()
        pool_res.release()
        pool_at.release()
        pool_v1.release()
        pool_kq.release()
        pool_xq.release()
        pool_w1s.release()
        pool_pT.release()
        pool_wD.release()
        pool_w2.release()
        psE.release()
        es0.close()

    nc.compile()
    return nc


# ---------------------------------------------------------------------------
# Host side
# ---------------------------------------------------------------------------

_PROGRAM_CACHE = {}


def _get_program(kpt, apply_ln1, apply_ln2, sim_compat=False, dbg=False):
    key = (kpt, apply_ln1, apply_ln2, sim_compat, dbg)
    if key not in _PROGRAM_CACHE:
        _PROGRAM_CACHE[key] = build_program(*key)
    return _PROGRAM_CACHE[key]


def _prep_inputs(x, mask, freqs_cos, freqs_sin, wq, wk, wv, wo, w1, w2,
                 ln1_w, ln1_b, ln2_w, ln2_b):
    """Build the 8 per-core input dicts.  Returns (kpt, in_maps)."""
    f32 = np.float32
    fp8 = ml_dtypes.float8_e4m3
    x = np.asarray(x, f32)
    mask = np.asarray(mask)
    cos = np.asarray(freqs_cos, f32)
    sin = np.asarray(freqs_sin, f32)
    B = x.shape[0]

    # interleaved +-, pre-scaled rope tables for all TB positions
    ci = np.empty((TB, HD), f32)
    ci[:, 0::2] = cos
    ci[:, 1::2] = cos
    si = np.empty((TB, HD), f32)
    si[:, 0::2] = -sin
    si[:, 1::2] = sin
    ci *= RWS
    si *= RWS

    idxs = [np.where(~mask[b])[0] for b in range(B)]
    kmax = max(1, max(len(ix) for ix in idxs))
    kpt = -(-kmax // P)          # ceil tiles
    kpt += kpt & 1               # even for DR pairing
    kpt = min(max(kpt, 2), 8)
    KP = kpt * P

    def tok_layout(t):
        return np.ascontiguousarray(
            t.reshape(-1, P, t.shape[-1] if t.ndim > 1 else 1)
            .transpose(1, 0, 2))

    def wlayout(w, kc):
        w = np.asarray(w, f32)
        return np.ascontiguousarray(
            (w * WS).reshape(kc, P, w.shape[1]).transpose(1, 0, 2)).astype(fp8)

    # w1a [P, FB, KC, P] bf16, fb-major: w1a[p, fb, c, t] = w1[c*128+p, fb*128+t]
    w1f = np.asarray(w1, f32).reshape(KC, P, FB, P)
    w1a = np.ascontiguousarray(w1f.transpose(1, 2, 0, 3)).astype(
        ml_dtypes.bfloat16)

    shared = {
        "wq": wlayout(wq, KC), "wk": wlayout(wk, KC), "wv": wlayout(wv, KC),
        "wo": wlayout(wo, KC),
        "w1a": w1a,
        "w2": wlayout(w2, FF // P),
        "ln1w": np.asarray(ln1_w, f32).reshape(1, D),
        "ln1b": np.asarray(ln1_b, f32).reshape(1, D),
        "ln2w": np.asarray(ln2_w, f32).reshape(1, D),
        "ln2b": np.asarray(ln2_b, f32).reshape(1, D),
    }

    in_maps = []
    for c in range(NCORES):
        b, half = divmod(c, 2)
        ix = idxs[b]
        npad = KP - len(ix)
        ixp = np.concatenate([ix, np.zeros(npad, np.int64)])
        mb = np.concatenate([np.zeros(len(ix), f32),
                             np.full(npad, MBIAS, f32)])
        lo = half * TL
        m = dict(shared)
        m["xq"] = np.ascontiguousarray(x[b, lo:lo + TL])
        m["xk"] = np.ascontiguousarray(x[b][ixp])
        m["mb"] = np.ascontiguousarray(tok_layout(mb)[:, :, 0])
        m["cosq"] = tok_layout(ci[lo:lo + TL])
        m["sinq"] = tok_layout(si[lo:lo + TL])
        m["cosk"] = tok_layout(ci[ixp])
        m["sink"] = tok_layout(si[ixp])
        in_maps.append(m)
    return kpt, in_maps


def kernel(x, mask, freqs_cos, freqs_sin, wq, wk, wv, wo, w1, w2,
           ln1_w, ln1_b, ln2_w, ln2_b, _trace=False, _sim=False, _dbg=False):
    from concourse.bass_utils import run_bass_kernel_spmd

    apply_ln1 = not (np.all(np.asarray(ln1_w) == 1.0)
                     and np.all(np.asarray(ln1_b) == 0.0))
    apply_ln2 = not (np.all(np.asarray(ln2_w) == 1.0)
                     and np.all(np.asarray(ln2_b) == 0.0))
    kpt, in_maps = _prep_inputs(x, mask, freqs_cos, freqs_sin, wq, wk, wv,
                                wo, w1, w2, ln1_w, ln1_b, ln2_w, ln2_b)
    nc = _get_program(kpt, apply_ln1, apply_ln2,
                      sim_compat=(_sim is not False and _sim is not None),
                      dbg=_dbg)

    if _sim is not False and _sim is not None:
        cidx = 0 if _sim is True else int(_sim)
        from concourse.bass_interp import CoreSim
        sim = CoreSim(nc, trace=False)
        for k, v in in_maps[cidx].items():
            sim.tensor(k)[:] = v
        sim.simulate(check_with_hw=False)
        b, half = divmod(cidx, 2)
        full = np.zeros((4, TB, D), np.float32)
        full[b, half * TL:(half + 1) * TL] = np.array(sim.tensor("out"))
        if _dbg:
            return full, sim
        return full

    res = run_bass_kernel_spmd(nc, in_maps, core_ids=list(range(NCORES)),
                               trace=_trace)
    full = np.empty((4, TB, D), np.float32)
    for c in range(NCORES):
        b, half = divmod(c, 2)
        full[b, half * TL:(half + 1) * TL] = res.results[c]["out"]
    if _trace:
        return full, res
    return full
